# revision 3
# baseline (speedup 1.0000x reference)
"""DirGCNConv on 8 Trainium2 NeuronCores via Bass/Tile (v3: 4-queue gather).

out = (1-a)*(Dout^-1/2 A Din^-1/2 x) @ Wsrc.T + a*(Din^-1/2 A.T Dout^-1/2 x) @ Wdst.T + bias

v3 vs v2:
- dma_gather calls rotate over 4 SWDGE queues; descriptor generation for the
  4 queues runs on 4 distinct Q7 CPU pairs concurrently -> ~4x gather rate
  (2.2ns/idx vs 8.6), removing the GpSimd bottleneck.
- Gathers read raw fp32 x directly (no prescale pass, no bf16 xb/xa DRAM
  round trip): the per-source deg^-1/2 scale is applied to the gathered
  tokens (tokens live on partitions) fused with the fp32->bf16 cast on DVE.
  All degree-derived scales are precomputed host-side (pure edge_index
  metadata, like the rp arrays v2 shipped).
- Dest blocks (128 rows) are assigned to cores by balanced snake dealing and
  sorted within cores, so the SPMD static schedule's max-over-cores padding
  drops from ~12% to ~2%.
"""

import os

import numpy as np
from contextlib import ExitStack

os.environ.setdefault("NEURON_RT_RESET_CORES", "1")

N = 100000
E = 600000
D = 128
NCORES = 8
ALPHA = 0.5

WBOUNDS = [0, 25000, 50000, 75000, 100000]
CALL = 1024          # max tokens per dma_gather call
NQ = 4               # SWDGE queues
NBLK = 98            # dest-block slots per core (8*98=784 >= ceil(N/128)=782)
GBLK = (N + 127) // 128   # 782 real global blocks


def _cfg_for(n_nodes):
    assert n_nodes == WBOUNDS[-1]
    return dict(N=n_nodes, NW=len(WBOUNDS) - 1, NLOC=NBLK * 128, NBLK=NBLK)


def _wrap_idx(arr):
    b = arr.shape[0]
    assert b % 16 == 0
    t = arr.reshape(b // 16, 16).T.copy()
    return np.tile(t, (8, 1)).astype(np.int16)


def _inv_sqrt_np(d):
    return np.where(d > 0, 1.0 / np.sqrt(np.maximum(d, 1.0)), 0.0).astype(
        np.float32)


def _prep_host(x, edge_index, W_src, b_src, W_dst, b_dst, cfg):
    """Index reorganization + degree metadata on host."""
    nw, nblk = cfg["NW"], cfg["NBLK"]
    row = np.asarray(edge_index[0], dtype=np.int64)
    col = np.asarray(edge_index[1], dtype=np.int64)

    deg_out = np.bincount(row, minlength=N).astype(np.float64)
    deg_in = np.bincount(col, minlength=N).astype(np.float64)
    a_full = _inv_sqrt_np(deg_out)      # scale at row endpoint
    b_full = _inv_sqrt_np(deg_in)       # scale at col endpoint

    # --- balanced assignment of global dest blocks to (core, slot) ---
    blk_sz = (np.bincount(row >> 7, minlength=nblk * NCORES)
              + np.bincount(col >> 7, minlength=nblk * NCORES))
    order = np.argsort(-blk_sz, kind="stable")
    blocks = [[] for _ in range(NCORES)]
    for i, g in enumerate(order):
        r = i // NCORES
        c = i % NCORES if r % 2 == 0 else NCORES - 1 - (i % NCORES)
        blocks[c].append(int(g))
    for c in range(NCORES):
        blocks[c].sort(key=lambda g: -blk_sz[g])
        assert len(blocks[c]) == nblk
    # map: global block -> (core, slot)
    blk_core = np.empty(nblk * NCORES, np.int64)
    blk_slot = np.empty(nblk * NCORES, np.int64)
    for c in range(NCORES):
        for s, g in enumerate(blocks[c]):
            blk_core[g] = c
            blk_slot[g] = s

    def bucket(dest, src, sfull):
        """dest-sorted token streams. Returns (plan, per-core arrays)."""
        g_of_d = dest >> 7
        core = blk_core[g_of_d]
        pc = []
        cnt = np.zeros((NCORES, nw, nblk), np.int64)
        for c in range(NCORES):
            m = core == c
            s = src[m].astype(np.int64)
            slot = blk_slot[g_of_d[m]]
            dl = slot * 128 + (dest[m] & 127)      # core-local dest id
            w = np.searchsorted(WBOUNDS, s, side="right") - 1
            o = np.lexsort((dl, slot, w))
            dl, s, w, slot = dl[o], s[o], w[o], slot[o]
            np.add.at(cnt[c], (w, slot), 1)
            pc.append((dl, s, w, slot))
        size_wb = cnt.max(axis=0)                      # [nw, nblk] static
        starts = np.zeros((nw, nblk + 1), np.int64)
        starts[:, 1:] = np.cumsum(size_wb, axis=1)
        wtot = starts[:, -1]
        ntokw = ((wtot + 127) // 128) * 128
        win_tok0 = np.zeros(nw, np.int64)
        win_tok0[1:] = np.cumsum(ntokw)[:-1]
        total = int(ntokw.sum())
        nch_total = total // 128

        windows = []
        for w in range(nw):
            nch = int(ntokw[w]) // 128
            mms = []
            for ci in range(nch):
                lo_t, hi_t = ci * 128, ci * 128 + 128
                for b in range(nblk):
                    if size_wb[w, b] > 0 and starts[w, b] < hi_t \
                            and starts[w, b + 1] > lo_t:
                        mms.append((ci, b))
            calls = [(a, min(CALL, int(ntokw[w]) - a))
                     for a in range(0, int(ntokw[w]), CALL)]
            seg_first, seg_last = {}, {}
            for j, (ci, b) in enumerate(mms):
                seg_first.setdefault(b, j)
                seg_last[b] = j
            windows.append(dict(tok0=int(win_tok0[w]), ntok=int(ntokw[w]),
                                calls=calls, mms=mms,
                                seg_first=seg_first, seg_last=seg_last))

        nmm = sum(len(wd["mms"]) for wd in windows)
        g_list, dl_list, st_list = [], [], []
        for c in range(NCORES):
            dl, s, w, slot = pc[c]
            key = w * nblk + slot
            gs0 = np.r_[0, np.cumsum(np.bincount(key, minlength=nw * nblk))]
            rank = np.arange(len(dl)) - gs0[key]
            pos = win_tok0[w] + starts[w, slot] + rank
            gfull = np.zeros(total, np.int64)          # global src (pad 0)
            dlv = -np.ones(total, np.int64)
            stok = np.zeros(total, np.float32)         # pad tokens scale 0
            gfull[pos] = s
            dlv[pos] = dl
            stok[pos] = sfull[s]
            # window-local int16 gather indices
            gloc = gfull.copy()
            for w2 in range(nw):
                t0, nt = windows[w2]["tok0"], windows[w2]["ntok"]
                gloc[t0:t0 + nt] -= WBOUNDS[w2]
                gloc[t0:t0 + nt] = np.maximum(gloc[t0:t0 + nt], 0)
            cols = np.empty((nmm, 128), np.int16)
            j = 0
            for w2, wd in enumerate(windows):
                dlw = dlv[wd["tok0"]:wd["tok0"] + wd["ntok"]].reshape(-1, 128)
                for (ci, b2) in wd["mms"]:
                    r = dlw[ci]
                    cols[j] = np.where((r >= b2 * 128) & (r < (b2 + 1) * 128),
                                       r - b2 * 128, -1).astype(np.int16)
                    j += 1
            g_list.append(_wrap_idx(gloc.astype(np.int16)))
            dl_list.append(np.ascontiguousarray(cols.T))       # [128, nmm]
            st_list.append(np.ascontiguousarray(
                stok.reshape(nch_total, 128).T))               # [128, nch]
        return (dict(windows=windows, total=total, nmm=nmm,
                     nch=nch_total), g_list, dl_list, st_list)

    plan1, g1, dl1, st1 = bucket(row, col, b_full)   # agg1[row] += b[col]x[col]
    plan2, g2, dl2, st2 = bucket(col, row, a_full)   # agg2[col] += a[row]x[row]

    # dest-side scales per (core, slot): avec for dir1 (a at dest row),
    # bvec for dir2 (b at dest col); scaled by alpha weights.
    av = np.zeros((NCORES, 128, nblk), np.float32)
    bv = np.zeros((NCORES, 128, nblk), np.float32)
    af_pad = np.r_[a_full, np.zeros(nblk * NCORES * 128 - N, np.float32)]
    bf_pad = np.r_[b_full, np.zeros(nblk * NCORES * 128 - N, np.float32)]
    for c in range(NCORES):
        for s, g in enumerate(blocks[c]):
            av[c, :, s] = (1.0 - ALPHA) * af_pad[g * 128:(g + 1) * 128]
            bv[c, :, s] = ALPHA * bf_pad[g * 128:(g + 1) * 128]

    wsrcT = np.ascontiguousarray(np.asarray(W_src, np.float32).T)
    wdstT = np.ascontiguousarray(np.asarray(W_dst, np.float32).T)
    xf = np.ascontiguousarray(np.asarray(x, np.float32))
    iota = np.tile(np.arange(128, dtype=np.float32)[None, :], (128, 1))

    in_maps = []
    for c in range(NCORES):
        in_maps.append({
            "x": xf, "wsrcT": wsrcT, "wdstT": wdstT, "iota": iota,
            "bsrc": np.asarray(b_src, np.float32),
            "bdst": np.asarray(b_dst, np.float32),
            "g1": g1[c], "g2": g2[c],
            "dl1": dl1[c], "dl2": dl2[c],
            "st1": st1[c], "st2": st2[c],
            "avec": av[c], "bvec": bv[c],
        })
    return in_maps, plan1, plan2, blocks


def _build(cfg, plan1, plan2):
    import concourse.tile as tile
    from concourse import bacc, mybir

    dt = mybir.dt
    n, nw = cfg["N"], cfg["NW"]
    nloc, nblk = cfg["NLOC"], cfg["NBLK"]

    nc = bacc.Bacc("TRN2", target_bir_lowering=False, debug=False,
                   num_devices=NCORES, num_swdge_queues=NQ)

    x = nc.dram_tensor("x", [n, D], dt.float32, kind="ExternalInput")
    wsrcT = nc.dram_tensor("wsrcT", [D, D], dt.float32, kind="ExternalInput")
    wdstT = nc.dram_tensor("wdstT", [D, D], dt.float32, kind="ExternalInput")
    iota = nc.dram_tensor("iota", [D, D], dt.float32, kind="ExternalInput")
    bsrc = nc.dram_tensor("bsrc", [D], dt.float32, kind="ExternalInput")
    bdst = nc.dram_tensor("bdst", [D], dt.float32, kind="ExternalInput")
    g1 = nc.dram_tensor("g1", [128, plan1["total"] // 16], dt.int16,
                        kind="ExternalInput")
    g2 = nc.dram_tensor("g2", [128, plan2["total"] // 16], dt.int16,
                        kind="ExternalInput")
    dl1 = nc.dram_tensor("dl1", [128, plan1["nmm"]], dt.int16,
                         kind="ExternalInput")
    dl2 = nc.dram_tensor("dl2", [128, plan2["nmm"]], dt.int16,
                         kind="ExternalInput")
    st1 = nc.dram_tensor("st1", [128, plan1["nch"]], dt.float32,
                         kind="ExternalInput")
    st2 = nc.dram_tensor("st2", [128, plan2["nch"]], dt.float32,
                         kind="ExternalInput")
    avec = nc.dram_tensor("avec", [128, nblk], dt.float32,
                          kind="ExternalInput")
    bvec = nc.dram_tensor("bvec", [128, nblk], dt.float32,
                          kind="ExternalInput")
    out = nc.dram_tensor("out", [nloc, D], dt.float32, kind="ExternalOutput")

    qcnt = [0]

    with tile.TileContext(nc) as tc, ExitStack() as ctx:
        const = ctx.enter_context(tc.tile_pool(name="const", bufs=1))

        wsrcT_sb = const.tile([D, D], dt.float32, tag="wsrc")
        nc.sync.dma_start(wsrcT_sb[:], wsrcT.ap())
        wdstT_sb = const.tile([D, D], dt.float32, tag="wdst")
        nc.sync.dma_start(wdstT_sb[:], wdstT.ap())
        iota_f = const.tile([D, D], dt.float32, tag="iotaf")
        nc.sync.dma_start(iota_f[:], iota.ap())
        iota_bf = const.tile([D, D], dt.bfloat16, tag="iota")
        nc.vector.tensor_copy(iota_bf[:], iota_f[:])

        brow = const.tile([1, 2 * D], dt.float32, tag="brow")
        nc.sync.dma_start(brow[:, 0:D], bsrc.ap().unsqueeze(0))
        nc.sync.dma_start(brow[:, D:2 * D], bdst.ap().unsqueeze(0))
        bsum = const.tile([1, D], dt.float32, tag="bsum")
        nc.vector.tensor_scalar_mul(bsum[:], brow[:, 0:D], 1.0 - ALPHA)
        bs2 = const.tile([1, D], dt.float32, tag="bs2")
        nc.vector.tensor_scalar_mul(bs2[:], brow[:, D:2 * D], ALPHA)
        nc.vector.tensor_add(bsum[:], bsum[:], bs2[:])
        bias_bc = const.tile([D, D], dt.float32, tag="biasbc")
        nc.gpsimd.partition_broadcast(bias_bc[:], bsum[:])

        g1_sb = const.tile([128, plan1["total"] // 16], dt.int16, tag="g1")
        nc.sync.dma_start(g1_sb[:], g1.ap())
        g2_sb = const.tile([128, plan2["total"] // 16], dt.int16, tag="g2")
        nc.sync.dma_start(g2_sb[:], g2.ap())

        st1_sb = const.tile([128, plan1["nch"]], dt.float32, tag="st1")
        nc.sync.dma_start(st1_sb[:], st1.ap())
        st2_sb = const.tile([128, plan2["nch"]], dt.float32, tag="st2")
        nc.sync.dma_start(st2_sb[:], st2.ap())
        av_sb = const.tile([128, nblk], dt.float32, tag="av")
        nc.sync.dma_start(av_sb[:], avec.ap())
        bv_sb = const.tile([128, nblk], dt.float32, tag="bv")
        nc.sync.dma_start(bv_sb[:], bvec.ap())

        dlf1 = const.tile([128, plan1["nmm"]], dt.bfloat16, tag="dlf1")
        dlf2 = const.tile([128, plan2["nmm"]], dt.bfloat16, tag="dlf2")
        with tc.tile_pool(name="chainscratch", bufs=1) as csp:
            di1 = csp.tile([128, plan1["nmm"]], dt.int16, tag="di1")
            nc.sync.dma_start(di1[:], dl1.ap())
            nc.vector.tensor_copy(dlf1[:], di1[:])
            di2 = csp.tile([128, plan2["nmm"]], dt.int16, tag="di2")
            nc.sync.dma_start(di2[:], dl2.ap())
            nc.vector.tensor_copy(dlf2[:], di2[:])

        # SBUF accumulators [feat x dest], one per direction
        agg1_sb = const.tile([128, nblk * 128], dt.float32, tag="agg1")
        agg2_sb = const.tile([128, nblk * 128], dt.float32, tag="agg2")
        for agg in (agg1_sb, agg2_sb):
            off = 0
            while off < nblk * 128:
                csz = min(4096, nblk * 128 - off)
                nc.vector.memset(agg[:, off:off + csz], 0.0)
                off += csz

        gpool = ctx.enter_context(tc.tile_pool(name="gat", bufs=4))
        xspool = ctx.enter_context(tc.tile_pool(name="xs", bufs=4))
        spool = ctx.enter_context(tc.tile_pool(name="sb", bufs=6))
        epsum = ctx.enter_context(tc.tile_pool(name="eps", bufs=4,
                                               space="PSUM"))

        def edge_window(plan, w, g_sb, st_sb, dlf, agg_sb, mm0, dtag,
                        final_cb=None):
            wd = plan["windows"][w]
            wbase = WBOUNDS[w]
            xs_src = x.ap()[wbase:WBOUNDS[w + 1], :]
            ch0 = wd["tok0"] // 128
            tiles = {}
            for (a, ln) in wd["calls"]:
                xt = gpool.tile([128, CALL // 128, D], dt.float32,
                                tag="xt" + dtag)
                o = wd["tok0"] + a
                gi = g_sb[:, o // 16:(o + ln) // 16]
                nc.gpsimd.dma_gather(xt[:, 0:ln // 128, :], xs_src, gi,
                                     ln, ln, D, queue_num=qcnt[0] % NQ)
                qcnt[0] += 1
                # fused per-token scale + fp32->bf16 cast
                xs = xspool.tile([128, CALL // 128, D], dt.bfloat16,
                                 tag="xs" + dtag)
                c0 = ch0 + a // 128
                nc.vector.tensor_tensor(
                    xs[:, 0:ln // 128, :],
                    st_sb[:, c0:c0 + ln // 128].unsqueeze(2).to_broadcast(
                        [128, ln // 128, D]),
                    xt[:, 0:ln // 128, :], mybir.AluOpType.mult)
                for g in range(ln // 128):
                    tiles[a // 128 + g] = (xs, g)
            mms = wd["mms"]
            sb_tiles = []
            for j0 in range(0, len(mms), 8):
                jn = min(8, len(mms) - j0)
                st = spool.tile([128, 8, D], dt.bfloat16, tag="st")
                nc.vector.tensor_tensor(
                    st[:, 0:jn, :],
                    dlf[:, mm0 + j0:mm0 + j0 + jn].unsqueeze(2)
                    .to_broadcast([128, jn, D]),
                    iota_bf[:].unsqueeze(1).to_broadcast([128, jn, D]),
                    mybir.AluOpType.is_equal)
                sb_tiles.append(st)
            active = {}
            for j, (ci, b) in enumerate(mms):
                xs, g = tiles[ci]
                st = sb_tiles[j // 8]
                if b not in active:
                    active[b] = epsum.tile([128, D], dt.float32, tag="ep",
                                           name="ep")
                ps = active[b]
                nc.tensor.matmul(ps[:], lhsT=xs[:, g, :], rhs=st[:, j % 8, :],
                                 start=(j == wd["seg_first"][b]),
                                 stop=(j == wd["seg_last"][b]))
                if j == wd["seg_last"][b]:
                    nc.vector.tensor_add(
                        agg_sb[:, b * 128:(b + 1) * 128],
                        agg_sb[:, b * 128:(b + 1) * 128], ps[:])
                    del active[b]
                    if final_cb is not None:
                        final_cb(b)
            return mm0 + len(mms)

        fp = ctx.enter_context(tc.tile_pool(name="fin", bufs=3))
        fps = ctx.enter_context(tc.tile_pool(name="fps", bufs=2,
                                             space="PSUM"))

        def final_block(k):
            ks = slice(k * 128, (k + 1) * 128)
            p1 = fps.tile([128, D], dt.float32, tag="p1", name="p1")
            nc.tensor.matmul(p1[:], lhsT=agg1_sb[:, ks], rhs=wsrcT_sb[:],
                             start=True, stop=True)
            p2 = fps.tile([128, D], dt.float32, tag="p2", name="p2")
            nc.tensor.matmul(p2[:], lhsT=agg2_sb[:, ks], rhs=wdstT_sb[:],
                             start=True, stop=True)
            o1 = fp.tile([128, D], dt.float32, tag="o1", name="o1")
            nc.scalar.mul(o1[:], p1[:], av_sb[:, k:k + 1])
            o2 = fp.tile([128, D], dt.float32, tag="o2", name="o2")
            nc.scalar.mul(o2[:], p2[:], bv_sb[:, k:k + 1])
            fin = fp.tile([128, D], dt.float32, tag="fin", name="fin")
            nc.vector.tensor_add(fin[:], o1[:], o2[:])
            nc.vector.tensor_add(fin[:], fin[:], bias_bc[:])
            nc.sync.dma_start(out.ap()[k * 128:(k + 1) * 128, :], fin[:])

        emitted = set()

        def final_cb(b):
            if b not in emitted:
                emitted.add(b)
                final_block(b)

        mm1, mm2 = 0, 0
        for w in range(nw):
            cb1 = final_cb if w == nw - 1 else None
            mm1 = edge_window(plan1, w, g1_sb, st1_sb, dlf1, agg1_sb, mm1,
                              "1", None)
            mm2 = edge_window(plan2, w, g2_sb, st2_sb, dlf2, agg2_sb, mm2,
                              "2", cb1)
        for k in range(nblk):
            if k not in emitted:
                emitted.add(k)
                final_block(k)

    nc.compile()
    return nc


def _install_ntff_shim():
    """This image's antenv lacks axon_hooks; inject it so trace=True works."""
    import sys
    import types
    try:
        from antenv import axon_hooks  # noqa: F401
        return
    except ImportError:
        pass
    try:
        import antenv
        from trn_agent_boot.trn_boot import _ntff_profile_via_ctypes
        mod = types.ModuleType("antenv.axon_hooks")
        holder = [None]
        mod.set_axon_ntff_profile_hook = lambda h: holder.__setitem__(0, h)
        mod.get_axon_ntff_profile_hook = lambda: holder[0]
        sys.modules["antenv.axon_hooks"] = mod
        antenv.axon_hooks = mod
        mod.set_axon_ntff_profile_hook(
            _ntff_profile_via_ctypes("/opt/axon/libaxon_pjrt.so"))
    except Exception as e:  # profiling is best-effort
        print("ntff shim failed:", e)


def _run(nc, in_maps, trace=False):
    from concourse.bass_utils import run_bass_kernel_spmd
    kw = {}
    if trace:
        _install_ntff_shim()
        kw = dict(trace=True, trace_cores=list(range(NCORES)))
    return run_bass_kernel_spmd(nc, in_maps, list(range(NCORES)), **kw)


def kernel(x, edge_index, W_src, b_src, W_dst, b_dst, _trace=False,
           _return_result=False):
    cfg = _cfg_for(x.shape[0])
    in_maps, plan1, plan2, blocks = _prep_host(
        x, edge_index, W_src, b_src, W_dst, b_dst, cfg)
    nc = _build(cfg, plan1, plan2)
    res = _run(nc, in_maps, trace=_trace)
    out = np.zeros((N, D), np.float32)
    for c in range(NCORES):
        oc = res.results[c]["out"]
        for s, g in enumerate(blocks[c]):
            lo = g * 128
            hi = min(lo + 128, N)
            if lo < N:
                out[lo:hi] = oc[s * 128:s * 128 + (hi - lo)]
    if _return_result:
        return out, res
    return out


# revision 4
# speedup vs baseline: 1.0834x; 1.0834x over previous
"""DirGCNConv on 8 Trainium2 NeuronCores via Bass/Tile (v3: 4-queue gather).

out = (1-a)*(Dout^-1/2 A Din^-1/2 x) @ Wsrc.T + a*(Din^-1/2 A.T Dout^-1/2 x) @ Wdst.T + bias

v3 vs v2:
- dma_gather calls rotate over 4 SWDGE queues; descriptor generation for the
  4 queues runs on 4 distinct Q7 CPU pairs concurrently -> ~4x gather rate
  (2.2ns/idx vs 8.6), removing the GpSimd bottleneck.
- Gathers read raw fp32 x directly (no prescale pass, no bf16 xb/xa DRAM
  round trip): the per-source deg^-1/2 scale is applied to the gathered
  tokens (tokens live on partitions) fused with the fp32->bf16 cast on DVE.
  All degree-derived scales are precomputed host-side (pure edge_index
  metadata, like the rp arrays v2 shipped).
- Dest blocks (128 rows) are assigned to cores by balanced snake dealing and
  sorted within cores, so the SPMD static schedule's max-over-cores padding
  drops from ~12% to ~2%.
"""

import os

import numpy as np
from contextlib import ExitStack

os.environ.setdefault("NEURON_RT_RESET_CORES", "1")

N = 100000
E = 600000
D = 128
NCORES = 8
ALPHA = 0.5

WBOUNDS = [0, 25000, 50000, 75000, 100000]
CALL = 1024          # max tokens per dma_gather call
NQ = 4               # SWDGE queues
NBLK = 98            # dest-block slots per core (8*98=784 >= ceil(N/128)=782)
GBLK = (N + 127) // 128   # 782 real global blocks


def _cfg_for(n_nodes):
    assert n_nodes == WBOUNDS[-1]
    return dict(N=n_nodes, NW=len(WBOUNDS) - 1, NLOC=NBLK * 128, NBLK=NBLK)


def _wrap_idx(arr):
    b = arr.shape[0]
    assert b % 16 == 0
    t = arr.reshape(b // 16, 16).T.copy()
    return np.tile(t, (8, 1)).astype(np.int16)


def _inv_sqrt_np(d):
    return np.where(d > 0, 1.0 / np.sqrt(np.maximum(d, 1.0)), 0.0).astype(
        np.float32)


def _prep_host(x, edge_index, W_src, b_src, W_dst, b_dst, cfg):
    """Index reorganization + degree metadata on host."""
    nw, nblk = cfg["NW"], cfg["NBLK"]
    row = np.asarray(edge_index[0], dtype=np.int64)
    col = np.asarray(edge_index[1], dtype=np.int64)

    deg_out = np.bincount(row, minlength=N).astype(np.float64)
    deg_in = np.bincount(col, minlength=N).astype(np.float64)
    a_full = _inv_sqrt_np(deg_out)      # scale at row endpoint
    b_full = _inv_sqrt_np(deg_in)       # scale at col endpoint

    # --- balanced assignment of global dest blocks to (core, slot) ---
    blk_sz = (np.bincount(row >> 7, minlength=nblk * NCORES)
              + np.bincount(col >> 7, minlength=nblk * NCORES))
    order = np.argsort(-blk_sz, kind="stable")
    blocks = [[] for _ in range(NCORES)]
    for i, g in enumerate(order):
        r = i // NCORES
        c = i % NCORES if r % 2 == 0 else NCORES - 1 - (i % NCORES)
        blocks[c].append(int(g))
    for c in range(NCORES):
        blocks[c].sort(key=lambda g: -blk_sz[g])
        assert len(blocks[c]) == nblk
    # map: global block -> (core, slot)
    blk_core = np.empty(nblk * NCORES, np.int64)
    blk_slot = np.empty(nblk * NCORES, np.int64)
    for c in range(NCORES):
        for s, g in enumerate(blocks[c]):
            blk_core[g] = c
            blk_slot[g] = s

    def bucket(dest, src, sfull):
        """dest-sorted token streams. Returns (plan, per-core arrays)."""
        g_of_d = dest >> 7
        core = blk_core[g_of_d]
        pc = []
        cnt = np.zeros((NCORES, nw, nblk), np.int64)
        for c in range(NCORES):
            m = core == c
            s = src[m].astype(np.int64)
            slot = blk_slot[g_of_d[m]]
            dl = slot * 128 + (dest[m] & 127)      # core-local dest id
            w = np.searchsorted(WBOUNDS, s, side="right") - 1
            o = np.lexsort((dl, slot, w))
            dl, s, w, slot = dl[o], s[o], w[o], slot[o]
            np.add.at(cnt[c], (w, slot), 1)
            pc.append((dl, s, w, slot))
        size_wb = cnt.max(axis=0)                      # [nw, nblk] static
        starts = np.zeros((nw, nblk + 1), np.int64)
        starts[:, 1:] = np.cumsum(size_wb, axis=1)
        wtot = starts[:, -1]
        ntokw = ((wtot + 127) // 128) * 128
        win_tok0 = np.zeros(nw, np.int64)
        win_tok0[1:] = np.cumsum(ntokw)[:-1]
        total = int(ntokw.sum())
        nch_total = total // 128

        windows = []
        for w in range(nw):
            nch = int(ntokw[w]) // 128
            mms = []
            for ci in range(nch):
                lo_t, hi_t = ci * 128, ci * 128 + 128
                for b in range(nblk):
                    if size_wb[w, b] > 0 and starts[w, b] < hi_t \
                            and starts[w, b + 1] > lo_t:
                        mms.append((ci, b))
            calls = [(a, min(CALL, int(ntokw[w]) - a))
                     for a in range(0, int(ntokw[w]), CALL)]
            seg_first, seg_last = {}, {}
            for j, (ci, b) in enumerate(mms):
                seg_first.setdefault(b, j)
                seg_last[b] = j
            windows.append(dict(tok0=int(win_tok0[w]), ntok=int(ntokw[w]),
                                calls=calls, mms=mms,
                                seg_first=seg_first, seg_last=seg_last))

        nmm = sum(len(wd["mms"]) for wd in windows)
        g_list, dl_list, st_list = [], [], []
        for c in range(NCORES):
            dl, s, w, slot = pc[c]
            key = w * nblk + slot
            gs0 = np.r_[0, np.cumsum(np.bincount(key, minlength=nw * nblk))]
            rank = np.arange(len(dl)) - gs0[key]
            pos = win_tok0[w] + starts[w, slot] + rank
            gfull = np.zeros(total, np.int64)          # global src (pad 0)
            dlv = -np.ones(total, np.int64)
            stok = np.zeros(total, np.float32)         # pad tokens scale 0
            gfull[pos] = s
            dlv[pos] = dl
            stok[pos] = sfull[s]
            # window-local int16 gather indices
            gloc = gfull.copy()
            for w2 in range(nw):
                t0, nt = windows[w2]["tok0"], windows[w2]["ntok"]
                gloc[t0:t0 + nt] -= WBOUNDS[w2]
                gloc[t0:t0 + nt] = np.maximum(gloc[t0:t0 + nt], 0)
            import ml_dtypes
            oh = np.zeros((nmm, 128, 128), dtype=np.float32)
            j = 0
            for w2, wd in enumerate(windows):
                dlw = dlv[wd["tok0"]:wd["tok0"] + wd["ntok"]].reshape(-1, 128)
                for (ci, b2) in wd["mms"]:
                    r = dlw[ci] - b2 * 128
                    m = (r >= 0) & (r < 128)
                    oh[j, np.arange(128)[m], r[m]] = 1.0
                    j += 1
            g_list.append(_wrap_idx(gloc.astype(np.int16)))
            dl_list.append(np.ascontiguousarray(
                oh.transpose(1, 0, 2).reshape(128, nmm * 128))
                .astype(ml_dtypes.float8_e4m3))    # [128, nmm*128]
            st_list.append(np.ascontiguousarray(
                stok.reshape(nch_total, 128).T))               # [128, nch]
        return (dict(windows=windows, total=total, nmm=nmm,
                     nch=nch_total), g_list, dl_list, st_list)

    plan1, g1, dl1, st1 = bucket(row, col, b_full)   # agg1[row] += b[col]x[col]
    plan2, g2, dl2, st2 = bucket(col, row, a_full)   # agg2[col] += a[row]x[row]

    # dest-side scales per (core, slot): avec for dir1 (a at dest row),
    # bvec for dir2 (b at dest col); scaled by alpha weights.
    av = np.zeros((NCORES, 128, nblk), np.float32)
    bv = np.zeros((NCORES, 128, nblk), np.float32)
    af_pad = np.r_[a_full, np.zeros(nblk * NCORES * 128 - N, np.float32)]
    bf_pad = np.r_[b_full, np.zeros(nblk * NCORES * 128 - N, np.float32)]
    for c in range(NCORES):
        for s, g in enumerate(blocks[c]):
            av[c, :, s] = (1.0 - ALPHA) * af_pad[g * 128:(g + 1) * 128]
            bv[c, :, s] = ALPHA * bf_pad[g * 128:(g + 1) * 128]

    wsrcT = np.ascontiguousarray(np.asarray(W_src, np.float32).T)
    wdstT = np.ascontiguousarray(np.asarray(W_dst, np.float32).T)
    xf = np.ascontiguousarray(np.asarray(x, np.float32))

    in_maps = []
    for c in range(NCORES):
        in_maps.append({
            "x": xf, "wsrcT": wsrcT, "wdstT": wdstT,
            "bsrc": np.asarray(b_src, np.float32),
            "bdst": np.asarray(b_dst, np.float32),
            "g1": g1[c], "g2": g2[c],
            "dl1": dl1[c], "dl2": dl2[c],
            "st1": st1[c], "st2": st2[c],
            "avec": av[c], "bvec": bv[c],
        })
    return in_maps, plan1, plan2, blocks


def _build(cfg, plan1, plan2):
    import concourse.tile as tile
    from concourse import bacc, mybir

    dt = mybir.dt
    n, nw = cfg["N"], cfg["NW"]
    nloc, nblk = cfg["NLOC"], cfg["NBLK"]

    nc = bacc.Bacc("TRN2", target_bir_lowering=False, debug=False,
                   num_devices=NCORES, num_swdge_queues=NQ)

    x = nc.dram_tensor("x", [n, D], dt.float32, kind="ExternalInput")
    wsrcT = nc.dram_tensor("wsrcT", [D, D], dt.float32, kind="ExternalInput")
    wdstT = nc.dram_tensor("wdstT", [D, D], dt.float32, kind="ExternalInput")
    bsrc = nc.dram_tensor("bsrc", [D], dt.float32, kind="ExternalInput")
    bdst = nc.dram_tensor("bdst", [D], dt.float32, kind="ExternalInput")
    g1 = nc.dram_tensor("g1", [128, plan1["total"] // 16], dt.int16,
                        kind="ExternalInput")
    g2 = nc.dram_tensor("g2", [128, plan2["total"] // 16], dt.int16,
                        kind="ExternalInput")
    dl1 = nc.dram_tensor("dl1", [128, plan1["nmm"] * 128], dt.float8e4,
                         kind="ExternalInput")
    dl2 = nc.dram_tensor("dl2", [128, plan2["nmm"] * 128], dt.float8e4,
                         kind="ExternalInput")
    st1 = nc.dram_tensor("st1", [128, plan1["nch"]], dt.float32,
                         kind="ExternalInput")
    st2 = nc.dram_tensor("st2", [128, plan2["nch"]], dt.float32,
                         kind="ExternalInput")
    avec = nc.dram_tensor("avec", [128, nblk], dt.float32,
                          kind="ExternalInput")
    bvec = nc.dram_tensor("bvec", [128, nblk], dt.float32,
                          kind="ExternalInput")
    out = nc.dram_tensor("out", [nloc, D], dt.float32, kind="ExternalOutput")

    qcnt = [0]

    with tile.TileContext(nc) as tc, ExitStack() as ctx:
        const = ctx.enter_context(tc.tile_pool(name="const", bufs=1))

        wsrcT_sb = const.tile([D, D], dt.float32, tag="wsrc")
        nc.sync.dma_start(wsrcT_sb[:], wsrcT.ap())
        wdstT_sb = const.tile([D, D], dt.float32, tag="wdst")
        nc.sync.dma_start(wdstT_sb[:], wdstT.ap())
        brow = const.tile([1, 2 * D], dt.float32, tag="brow")
        nc.sync.dma_start(brow[:, 0:D], bsrc.ap().unsqueeze(0))
        nc.sync.dma_start(brow[:, D:2 * D], bdst.ap().unsqueeze(0))
        bsum = const.tile([1, D], dt.float32, tag="bsum")
        nc.vector.tensor_scalar_mul(bsum[:], brow[:, 0:D], 1.0 - ALPHA)
        bs2 = const.tile([1, D], dt.float32, tag="bs2")
        nc.vector.tensor_scalar_mul(bs2[:], brow[:, D:2 * D], ALPHA)
        nc.vector.tensor_add(bsum[:], bsum[:], bs2[:])
        bias_bc = const.tile([D, D], dt.float32, tag="biasbc")
        nc.gpsimd.partition_broadcast(bias_bc[:], bsum[:])

        g1_sb = const.tile([128, plan1["total"] // 16], dt.int16, tag="g1")
        nc.sync.dma_start(g1_sb[:], g1.ap())
        g2_sb = const.tile([128, plan2["total"] // 16], dt.int16, tag="g2")
        nc.sync.dma_start(g2_sb[:], g2.ap())

        st1_sb = const.tile([128, plan1["nch"]], dt.float32, tag="st1")
        nc.sync.dma_start(st1_sb[:], st1.ap())
        st2_sb = const.tile([128, plan2["nch"]], dt.float32, tag="st2")
        nc.sync.dma_start(st2_sb[:], st2.ap())
        av_sb = const.tile([128, nblk], dt.float32, tag="av")
        nc.sync.dma_start(av_sb[:], avec.ap())
        bv_sb = const.tile([128, nblk], dt.float32, tag="bv")
        nc.sync.dma_start(bv_sb[:], bvec.ap())

        # SBUF accumulators [feat x dest], one per direction
        agg1_sb = const.tile([128, nblk * 128], dt.float32, tag="agg1")
        agg2_sb = const.tile([128, nblk * 128], dt.float32, tag="agg2")
        for agg in (agg1_sb, agg2_sb):
            off = 0
            while off < nblk * 128:
                csz = min(4096, nblk * 128 - off)
                nc.vector.memset(agg[:, off:off + csz], 0.0)
                off += csz

        gpool = ctx.enter_context(tc.tile_pool(name="gat", bufs=4))
        xspool = ctx.enter_context(tc.tile_pool(name="xs", bufs=4))
        spool = ctx.enter_context(tc.tile_pool(name="sb", bufs=6))
        epsum = ctx.enter_context(tc.tile_pool(name="eps", bufs=4,
                                               space="PSUM"))

        def edge_window(plan, w, g_sb, st_sb, dlt, agg_sb, mm0, dtag,
                        final_cb=None):
            wd = plan["windows"][w]
            wbase = WBOUNDS[w]
            xs_src = x.ap()[wbase:WBOUNDS[w + 1], :]
            ch0 = wd["tok0"] // 128
            tiles = {}
            for (a, ln) in wd["calls"]:
                xt = gpool.tile([128, CALL // 128, D], dt.float32,
                                tag="xt" + dtag)
                o = wd["tok0"] + a
                gi = g_sb[:, o // 16:(o + ln) // 16]
                nc.gpsimd.dma_gather(xt[:, 0:ln // 128, :], xs_src, gi,
                                     ln, ln, D, queue_num=qcnt[0] % NQ)
                qcnt[0] += 1
                # fused per-token scale + fp32->bf16 cast
                xs = xspool.tile([128, CALL // 128, D], dt.bfloat16,
                                 tag="xs" + dtag)
                c0 = ch0 + a // 128
                if qcnt[0] % 3 == 0:
                    for g in range(ln // 128):
                        nc.scalar.mul(xs[:, g, :], xt[:, g, :],
                                      st_sb[:, c0 + g:c0 + g + 1])
                else:
                    nc.vector.tensor_tensor(
                        xs[:, 0:ln // 128, :],
                        st_sb[:, c0:c0 + ln // 128].unsqueeze(2).to_broadcast(
                            [128, ln // 128, D]),
                        xt[:, 0:ln // 128, :], mybir.AluOpType.mult)
                for g in range(ln // 128):
                    tiles[a // 128 + g] = (xs, g)
            mms = wd["mms"]
            sb_tiles = []
            for j0 in range(0, len(mms), 8):
                jn = min(8, len(mms) - j0)
                st = spool.tile([128, 8, D], dt.float8e4, tag="st")
                nc.sync.dma_start(
                    st[:, 0:jn, :],
                    dlt.ap()[:, (mm0 + j0) * 128:(mm0 + j0 + jn) * 128]
                    .rearrange("p (j d) -> p j d", d=128))
                sb_tiles.append(st)
            active = {}
            for j, (ci, b) in enumerate(mms):
                xs, g = tiles[ci]
                st = sb_tiles[j // 8]
                if b not in active:
                    active[b] = epsum.tile([128, D], dt.float32, tag="ep",
                                           name="ep")
                ps = active[b]
                nc.tensor.matmul(ps[:], lhsT=xs[:, g, :], rhs=st[:, j % 8, :],
                                 start=(j == wd["seg_first"][b]),
                                 stop=(j == wd["seg_last"][b]))
                if j == wd["seg_last"][b]:
                    nc.vector.tensor_add(
                        agg_sb[:, b * 128:(b + 1) * 128],
                        agg_sb[:, b * 128:(b + 1) * 128], ps[:])
                    del active[b]
                    if final_cb is not None:
                        final_cb(b)
            return mm0 + len(mms)

        fp = ctx.enter_context(tc.tile_pool(name="fin", bufs=3))
        fps = ctx.enter_context(tc.tile_pool(name="fps", bufs=2,
                                             space="PSUM"))

        def final_block(k):
            ks = slice(k * 128, (k + 1) * 128)
            p1 = fps.tile([128, D], dt.float32, tag="p1", name="p1")
            nc.tensor.matmul(p1[:], lhsT=agg1_sb[:, ks], rhs=wsrcT_sb[:],
                             start=True, stop=True)
            p2 = fps.tile([128, D], dt.float32, tag="p2", name="p2")
            nc.tensor.matmul(p2[:], lhsT=agg2_sb[:, ks], rhs=wdstT_sb[:],
                             start=True, stop=True)
            o1 = fp.tile([128, D], dt.float32, tag="o1", name="o1")
            nc.scalar.mul(o1[:], p1[:], av_sb[:, k:k + 1])
            o2 = fp.tile([128, D], dt.float32, tag="o2", name="o2")
            nc.scalar.mul(o2[:], p2[:], bv_sb[:, k:k + 1])
            fin = fp.tile([128, D], dt.float32, tag="fin", name="fin")
            nc.vector.tensor_add(fin[:], o1[:], o2[:])
            nc.vector.tensor_add(fin[:], fin[:], bias_bc[:])
            nc.sync.dma_start(out.ap()[k * 128:(k + 1) * 128, :], fin[:])

        emitted = set()

        def final_cb(b):
            if b not in emitted:
                emitted.add(b)
                final_block(b)

        mm1, mm2 = 0, 0
        for w in range(nw):
            cb1 = final_cb if w == nw - 1 else None
            mm1 = edge_window(plan1, w, g1_sb, st1_sb, dl1, agg1_sb, mm1,
                              "1", None)
            mm2 = edge_window(plan2, w, g2_sb, st2_sb, dl2, agg2_sb, mm2,
                              "2", cb1)
        for k in range(nblk):
            if k not in emitted:
                emitted.add(k)
                final_block(k)

    nc.compile()
    return nc


def _install_ntff_shim():
    """This image's antenv lacks axon_hooks; inject it so trace=True works."""
    import sys
    import types
    try:
        from antenv import axon_hooks  # noqa: F401
        return
    except ImportError:
        pass
    try:
        import antenv
        from trn_agent_boot.trn_boot import _ntff_profile_via_ctypes
        mod = types.ModuleType("antenv.axon_hooks")
        holder = [None]
        mod.set_axon_ntff_profile_hook = lambda h: holder.__setitem__(0, h)
        mod.get_axon_ntff_profile_hook = lambda: holder[0]
        sys.modules["antenv.axon_hooks"] = mod
        antenv.axon_hooks = mod
        mod.set_axon_ntff_profile_hook(
            _ntff_profile_via_ctypes("/opt/axon/libaxon_pjrt.so"))
    except Exception as e:  # profiling is best-effort
        print("ntff shim failed:", e)


def _run(nc, in_maps, trace=False):
    from concourse.bass_utils import run_bass_kernel_spmd
    kw = {}
    if trace:
        _install_ntff_shim()
        kw = dict(trace=True, trace_cores=list(range(NCORES)))
    return run_bass_kernel_spmd(nc, in_maps, list(range(NCORES)), **kw)


def kernel(x, edge_index, W_src, b_src, W_dst, b_dst, _trace=False,
           _return_result=False):
    cfg = _cfg_for(x.shape[0])
    in_maps, plan1, plan2, blocks = _prep_host(
        x, edge_index, W_src, b_src, W_dst, b_dst, cfg)
    nc = _build(cfg, plan1, plan2)
    res = _run(nc, in_maps, trace=_trace)
    out = np.zeros((N, D), np.float32)
    for c in range(NCORES):
        oc = res.results[c]["out"]
        for s, g in enumerate(blocks[c]):
            lo = g * 128
            hi = min(lo + 128, N)
            if lo < N:
                out[lo:hi] = oc[s * 128:s * 128 + (hi - lo)]
    if _return_result:
        return out, res
    return out


# revision 6
# speedup vs baseline: 1.1332x; 1.0460x over previous
"""DirGCNConv on 8 Trainium2 NeuronCores via Bass/Tile (v3: 4-queue gather).

out = (1-a)*(Dout^-1/2 A Din^-1/2 x) @ Wsrc.T + a*(Din^-1/2 A.T Dout^-1/2 x) @ Wdst.T + bias

v3 vs v2:
- dma_gather calls rotate over 4 SWDGE queues; descriptor generation for the
  4 queues runs on 4 distinct Q7 CPU pairs concurrently -> ~4x gather rate
  (2.2ns/idx vs 8.6), removing the GpSimd bottleneck.
- Gathers read raw fp32 x directly (no prescale pass, no bf16 xb/xa DRAM
  round trip): the per-source deg^-1/2 scale is applied to the gathered
  tokens (tokens live on partitions) fused with the fp32->bf16 cast on DVE.
  All degree-derived scales are precomputed host-side (pure edge_index
  metadata, like the rp arrays v2 shipped).
- Dest blocks (128 rows) are assigned to cores by balanced snake dealing and
  sorted within cores, so the SPMD static schedule's max-over-cores padding
  drops from ~12% to ~2%.
"""

import os

import numpy as np
from contextlib import ExitStack

os.environ.setdefault("NEURON_RT_RESET_CORES", "1")

N = 100000
E = 600000
D = 128
NCORES = 8
ALPHA = 0.5

WBOUNDS = [0, 25000, 50000, 75000, 100000]
CALL = 1024          # max tokens per dma_gather call
NQ = 4               # SWDGE queues
NBLK = 98            # dest-block slots per core (8*98=784 >= ceil(N/128)=782)
GBLK = (N + 127) // 128   # 782 real global blocks


def _cfg_for(n_nodes):
    assert n_nodes == WBOUNDS[-1]
    return dict(N=n_nodes, NW=len(WBOUNDS) - 1, NLOC=NBLK * 128, NBLK=NBLK)


def _wrap_idx(arr):
    b = arr.shape[0]
    assert b % 16 == 0
    t = arr.reshape(b // 16, 16).T.copy()
    return np.tile(t, (8, 1)).astype(np.int16)


def _inv_sqrt_np(d):
    return np.where(d > 0, 1.0 / np.sqrt(np.maximum(d, 1.0)), 0.0).astype(
        np.float32)


def _prep_host(x, edge_index, W_src, b_src, W_dst, b_dst, cfg):
    """Index reorganization + degree metadata on host."""
    nw, nblk = cfg["NW"], cfg["NBLK"]
    row = np.asarray(edge_index[0], dtype=np.int64)
    col = np.asarray(edge_index[1], dtype=np.int64)

    deg_out = np.bincount(row, minlength=N).astype(np.float64)
    deg_in = np.bincount(col, minlength=N).astype(np.float64)
    a_full = _inv_sqrt_np(deg_out)      # scale at row endpoint
    b_full = _inv_sqrt_np(deg_in)       # scale at col endpoint

    # --- balanced assignment of global dest blocks to (core, slot) ---
    blk_sz = (np.bincount(row >> 7, minlength=nblk * NCORES)
              + np.bincount(col >> 7, minlength=nblk * NCORES))
    order = np.argsort(-blk_sz, kind="stable")
    blocks = [[] for _ in range(NCORES)]
    for i, g in enumerate(order):
        r = i // NCORES
        c = i % NCORES if r % 2 == 0 else NCORES - 1 - (i % NCORES)
        blocks[c].append(int(g))
    for c in range(NCORES):
        blocks[c].sort(key=lambda g: -blk_sz[g])
        assert len(blocks[c]) == nblk
    # map: global block -> (core, slot)
    blk_core = np.empty(nblk * NCORES, np.int64)
    blk_slot = np.empty(nblk * NCORES, np.int64)
    for c in range(NCORES):
        for s, g in enumerate(blocks[c]):
            blk_core[g] = c
            blk_slot[g] = s

    def bucket(dest, src, sfull):
        """dest-sorted token streams. Returns (plan, per-core arrays)."""
        g_of_d = dest >> 7
        core = blk_core[g_of_d]
        pc = []
        cnt = np.zeros((NCORES, nw, nblk), np.int64)
        for c in range(NCORES):
            m = core == c
            s = src[m].astype(np.int64)
            slot = blk_slot[g_of_d[m]]
            dl = slot * 128 + (dest[m] & 127)      # core-local dest id
            w = np.searchsorted(WBOUNDS, s, side="right") - 1
            o = np.lexsort((dl, slot, w))
            dl, s, w, slot = dl[o], s[o], w[o], slot[o]
            np.add.at(cnt[c], (w, slot), 1)
            pc.append((dl, s, w, slot))
        size_wb = cnt.max(axis=0)                      # [nw, nblk] static
        starts = np.zeros((nw, nblk + 1), np.int64)
        starts[:, 1:] = np.cumsum(size_wb, axis=1)
        wtot = starts[:, -1]
        ntokw = ((wtot + 127) // 128) * 128
        win_tok0 = np.zeros(nw, np.int64)
        win_tok0[1:] = np.cumsum(ntokw)[:-1]
        total = int(ntokw.sum())
        nch_total = total // 128

        windows = []
        for w in range(nw):
            nch = int(ntokw[w]) // 128
            mms = []
            for ci in range(nch):
                lo_t, hi_t = ci * 128, ci * 128 + 128
                for b in range(nblk):
                    if size_wb[w, b] > 0 and starts[w, b] < hi_t \
                            and starts[w, b + 1] > lo_t:
                        mms.append((ci, b))
            calls = [(a, min(CALL, int(ntokw[w]) - a))
                     for a in range(0, int(ntokw[w]), CALL)]
            seg_first, seg_last = {}, {}
            for j, (ci, b) in enumerate(mms):
                seg_first.setdefault(b, j)
                seg_last[b] = j
            windows.append(dict(tok0=int(win_tok0[w]), ntok=int(ntokw[w]),
                                calls=calls, mms=mms,
                                seg_first=seg_first, seg_last=seg_last))

        nmm = sum(len(wd["mms"]) for wd in windows)
        g_list, dl_list, st_list = [], [], []
        for c in range(NCORES):
            dl, s, w, slot = pc[c]
            key = w * nblk + slot
            gs0 = np.r_[0, np.cumsum(np.bincount(key, minlength=nw * nblk))]
            rank = np.arange(len(dl)) - gs0[key]
            pos = win_tok0[w] + starts[w, slot] + rank
            gfull = np.zeros(total, np.int64)          # global src (pad 0)
            dlv = -np.ones(total, np.int64)
            stok = np.zeros(total, np.float32)         # pad tokens scale 0
            gfull[pos] = s
            dlv[pos] = dl
            stok[pos] = sfull[s]
            # window-local int16 gather indices
            gloc = gfull.copy()
            for w2 in range(nw):
                t0, nt = windows[w2]["tok0"], windows[w2]["ntok"]
                gloc[t0:t0 + nt] -= WBOUNDS[w2]
                gloc[t0:t0 + nt] = np.maximum(gloc[t0:t0 + nt], 0)
            import ml_dtypes
            oh = np.zeros((nmm, 128, 128), dtype=np.float32)
            j = 0
            for w2, wd in enumerate(windows):
                dlw = dlv[wd["tok0"]:wd["tok0"] + wd["ntok"]].reshape(-1, 128)
                for (ci, b2) in wd["mms"]:
                    r = dlw[ci] - b2 * 128
                    m = (r >= 0) & (r < 128)
                    oh[j, np.arange(128)[m], r[m]] = 1.0
                    j += 1
            g_list.append(_wrap_idx(gloc.astype(np.int16)))
            dl_list.append(np.ascontiguousarray(
                oh.transpose(1, 0, 2).reshape(128, nmm * 128))
                .astype(ml_dtypes.float8_e4m3))    # [128, nmm*128]
            st_list.append(np.ascontiguousarray(
                stok.reshape(nch_total, 128).T))               # [128, nch]
        return (dict(windows=windows, total=total, nmm=nmm,
                     nch=nch_total), g_list, dl_list, st_list)

    plan1, g1, dl1, st1 = bucket(row, col, b_full)   # agg1[row] += b[col]x[col]
    plan2, g2, dl2, st2 = bucket(col, row, a_full)   # agg2[col] += a[row]x[row]

    # dest-side scales per (core, slot): avec for dir1 (a at dest row),
    # bvec for dir2 (b at dest col); scaled by alpha weights.
    av = np.zeros((NCORES, 128, nblk), np.float32)
    bv = np.zeros((NCORES, 128, nblk), np.float32)
    af_pad = np.r_[a_full, np.zeros(nblk * NCORES * 128 - N, np.float32)]
    bf_pad = np.r_[b_full, np.zeros(nblk * NCORES * 128 - N, np.float32)]
    for c in range(NCORES):
        for s, g in enumerate(blocks[c]):
            av[c, :, s] = (1.0 - ALPHA) * af_pad[g * 128:(g + 1) * 128]
            bv[c, :, s] = ALPHA * bf_pad[g * 128:(g + 1) * 128]

    wsrcT = np.ascontiguousarray(np.asarray(W_src, np.float32).T)
    wdstT = np.ascontiguousarray(np.asarray(W_dst, np.float32).T)
    xf = np.ascontiguousarray(np.asarray(x, np.float32))

    in_maps = []
    for c in range(NCORES):
        in_maps.append({
            "x": xf, "wsrcT": wsrcT, "wdstT": wdstT,
            "bsrc": np.asarray(b_src, np.float32),
            "bdst": np.asarray(b_dst, np.float32),
            "g1": g1[c], "g2": g2[c],
            "dl1": dl1[c], "dl2": dl2[c],
            "st1": st1[c], "st2": st2[c],
            "avec": av[c], "bvec": bv[c],
        })
    return in_maps, plan1, plan2, blocks


def _build(cfg, plan1, plan2):
    import concourse.tile as tile
    from concourse import bacc, mybir

    dt = mybir.dt
    n, nw = cfg["N"], cfg["NW"]
    nloc, nblk = cfg["NLOC"], cfg["NBLK"]

    nc = bacc.Bacc("TRN2", target_bir_lowering=False, debug=False,
                   num_devices=NCORES, num_swdge_queues=NQ)

    x = nc.dram_tensor("x", [n, D], dt.float32, kind="ExternalInput")
    wsrcT = nc.dram_tensor("wsrcT", [D, D], dt.float32, kind="ExternalInput")
    wdstT = nc.dram_tensor("wdstT", [D, D], dt.float32, kind="ExternalInput")
    bsrc = nc.dram_tensor("bsrc", [D], dt.float32, kind="ExternalInput")
    bdst = nc.dram_tensor("bdst", [D], dt.float32, kind="ExternalInput")
    g1 = nc.dram_tensor("g1", [128, plan1["total"] // 16], dt.int16,
                        kind="ExternalInput")
    g2 = nc.dram_tensor("g2", [128, plan2["total"] // 16], dt.int16,
                        kind="ExternalInput")
    dl1 = nc.dram_tensor("dl1", [128, plan1["nmm"] * 128], dt.float8e4,
                         kind="ExternalInput")
    dl2 = nc.dram_tensor("dl2", [128, plan2["nmm"] * 128], dt.float8e4,
                         kind="ExternalInput")
    st1 = nc.dram_tensor("st1", [128, plan1["nch"]], dt.float32,
                         kind="ExternalInput")
    st2 = nc.dram_tensor("st2", [128, plan2["nch"]], dt.float32,
                         kind="ExternalInput")
    avec = nc.dram_tensor("avec", [128, nblk], dt.float32,
                          kind="ExternalInput")
    bvec = nc.dram_tensor("bvec", [128, nblk], dt.float32,
                          kind="ExternalInput")
    out = nc.dram_tensor("out", [nloc, D], dt.float32, kind="ExternalOutput")

    qcnt = [0]

    with tile.TileContext(nc) as tc, ExitStack() as ctx:
        const = ctx.enter_context(tc.tile_pool(name="const", bufs=1))

        wsrcT_sb = const.tile([D, D], dt.float32, tag="wsrc")
        nc.sync.dma_start(wsrcT_sb[:], wsrcT.ap())
        wdstT_sb = const.tile([D, D], dt.float32, tag="wdst")
        nc.sync.dma_start(wdstT_sb[:], wdstT.ap())
        brow = const.tile([1, 2 * D], dt.float32, tag="brow")
        nc.sync.dma_start(brow[:, 0:D], bsrc.ap().unsqueeze(0))
        nc.sync.dma_start(brow[:, D:2 * D], bdst.ap().unsqueeze(0))
        bsum = const.tile([1, D], dt.float32, tag="bsum")
        nc.vector.tensor_scalar_mul(bsum[:], brow[:, 0:D], 1.0 - ALPHA)
        bs2 = const.tile([1, D], dt.float32, tag="bs2")
        nc.vector.tensor_scalar_mul(bs2[:], brow[:, D:2 * D], ALPHA)
        nc.vector.tensor_add(bsum[:], bsum[:], bs2[:])
        bias_bc = const.tile([D, D], dt.float32, tag="biasbc")
        nc.gpsimd.partition_broadcast(bias_bc[:], bsum[:])

        g1_sb = const.tile([128, plan1["total"] // 16], dt.int16, tag="g1")
        nc.sync.dma_start(g1_sb[:], g1.ap())
        g2_sb = const.tile([128, plan2["total"] // 16], dt.int16, tag="g2")
        nc.sync.dma_start(g2_sb[:], g2.ap())

        st1_sb = const.tile([128, plan1["nch"]], dt.float32, tag="st1")
        nc.sync.dma_start(st1_sb[:], st1.ap())
        st2_sb = const.tile([128, plan2["nch"]], dt.float32, tag="st2")
        nc.sync.dma_start(st2_sb[:], st2.ap())
        av_sb = const.tile([128, nblk], dt.float32, tag="av")
        nc.sync.dma_start(av_sb[:], avec.ap())
        bv_sb = const.tile([128, nblk], dt.float32, tag="bv")
        nc.sync.dma_start(bv_sb[:], bvec.ap())

        # SBUF accumulators [feat x dest], one per direction
        agg1_sb = const.tile([128, nblk * 128], dt.float32, tag="agg1")
        agg2_sb = const.tile([128, nblk * 128], dt.float32, tag="agg2")
        for agg in (agg1_sb, agg2_sb):
            off = 0
            while off < nblk * 128:
                csz = min(4096, nblk * 128 - off)
                nc.vector.memset(agg[:, off:off + csz], 0.0)
                off += csz

        gpool = ctx.enter_context(tc.tile_pool(name="gat", bufs=5))
        xspool = ctx.enter_context(tc.tile_pool(name="xs", bufs=6))
        spool = ctx.enter_context(tc.tile_pool(name="sb", bufs=8))
        epsum = ctx.enter_context(tc.tile_pool(name="eps", bufs=4,
                                               space="PSUM"))

        def edge_window(plan, w, g_sb, st_sb, dlt, agg_sb, mm0, dtag,
                        final_cb=None):
            wd = plan["windows"][w]
            wbase = WBOUNDS[w]
            xs_src = x.ap()[wbase:WBOUNDS[w + 1], :]
            ch0 = wd["tok0"] // 128
            tiles = {}
            for (a, ln) in wd["calls"]:
                xt = gpool.tile([128, CALL // 128, D], dt.float32,
                                tag="xt" + dtag)
                o = wd["tok0"] + a
                gi = g_sb[:, o // 16:(o + ln) // 16]
                nc.gpsimd.dma_gather(xt[:, 0:ln // 128, :], xs_src, gi,
                                     ln, ln, D, queue_num=qcnt[0] % NQ)
                qcnt[0] += 1
                # fused per-token scale + fp32->bf16 cast
                xs = xspool.tile([128, CALL // 128, D], dt.bfloat16,
                                 tag="xs" + dtag)
                c0 = ch0 + a // 128
                if qcnt[0] % 3 == 0:
                    for g in range(ln // 128):
                        nc.scalar.mul(xs[:, g, :], xt[:, g, :],
                                      st_sb[:, c0 + g:c0 + g + 1])
                else:
                    nc.vector.tensor_tensor(
                        xs[:, 0:ln // 128, :],
                        st_sb[:, c0:c0 + ln // 128].unsqueeze(2).to_broadcast(
                            [128, ln // 128, D]),
                        xt[:, 0:ln // 128, :], mybir.AluOpType.mult)
                for g in range(ln // 128):
                    tiles[a // 128 + g] = (xs, g)
            mms = wd["mms"]
            sb_tiles = []
            for j0 in range(0, len(mms), 8):
                jn = min(8, len(mms) - j0)
                st = spool.tile([128, 8, D], dt.float8e4, tag="st")
                nc.sync.dma_start(
                    st[:, 0:jn, :],
                    dlt.ap()[:, (mm0 + j0) * 128:(mm0 + j0 + jn) * 128]
                    .rearrange("p (j d) -> p j d", d=128))
                sb_tiles.append(st)
            active = {}
            for j, (ci, b) in enumerate(mms):
                xs, g = tiles[ci]
                st = sb_tiles[j // 8]
                if b not in active:
                    active[b] = epsum.tile([128, D], dt.float32, tag="ep",
                                           name="ep")
                ps = active[b]
                nc.tensor.matmul(ps[:], lhsT=xs[:, g, :], rhs=st[:, j % 8, :],
                                 start=(j == wd["seg_first"][b]),
                                 stop=(j == wd["seg_last"][b]))
                if j == wd["seg_last"][b]:
                    nc.vector.tensor_add(
                        agg_sb[:, b * 128:(b + 1) * 128],
                        agg_sb[:, b * 128:(b + 1) * 128], ps[:])
                    del active[b]
                    if final_cb is not None:
                        final_cb(b)
            return mm0 + len(mms)

        fp = ctx.enter_context(tc.tile_pool(name="fin", bufs=3))
        fps = ctx.enter_context(tc.tile_pool(name="fps", bufs=2,
                                             space="PSUM"))

        def final_block(k):
            ks = slice(k * 128, (k + 1) * 128)
            p1 = fps.tile([128, D], dt.float32, tag="p1", name="p1")
            nc.tensor.matmul(p1[:], lhsT=agg1_sb[:, ks], rhs=wsrcT_sb[:],
                             start=True, stop=True)
            p2 = fps.tile([128, D], dt.float32, tag="p2", name="p2")
            nc.tensor.matmul(p2[:], lhsT=agg2_sb[:, ks], rhs=wdstT_sb[:],
                             start=True, stop=True)
            o1 = fp.tile([128, D], dt.float32, tag="o1", name="o1")
            nc.scalar.mul(o1[:], p1[:], av_sb[:, k:k + 1])
            o2 = fp.tile([128, D], dt.float32, tag="o2", name="o2")
            nc.scalar.mul(o2[:], p2[:], bv_sb[:, k:k + 1])
            fin = fp.tile([128, D], dt.float32, tag="fin", name="fin")
            nc.vector.tensor_add(fin[:], o1[:], o2[:])
            nc.vector.tensor_add(fin[:], fin[:], bias_bc[:])
            nc.sync.dma_start(out.ap()[k * 128:(k + 1) * 128, :], fin[:])

        emitted = set()

        def final_cb(b):
            if b not in emitted:
                emitted.add(b)
                final_block(b)

        mm1, mm2 = 0, 0
        for w in range(nw):
            cb1 = final_cb if w == nw - 1 else None
            mm1 = edge_window(plan1, w, g1_sb, st1_sb, dl1, agg1_sb, mm1,
                              "1", None)
            mm2 = edge_window(plan2, w, g2_sb, st2_sb, dl2, agg2_sb, mm2,
                              "2", cb1)
        for k in range(nblk):
            if k not in emitted:
                emitted.add(k)
                final_block(k)

    nc.compile()
    return nc


def _install_ntff_shim():
    """This image's antenv lacks axon_hooks; inject it so trace=True works."""
    import sys
    import types
    try:
        from antenv import axon_hooks  # noqa: F401
        return
    except ImportError:
        pass
    try:
        import antenv
        from trn_agent_boot.trn_boot import _ntff_profile_via_ctypes
        mod = types.ModuleType("antenv.axon_hooks")
        holder = [None]
        mod.set_axon_ntff_profile_hook = lambda h: holder.__setitem__(0, h)
        mod.get_axon_ntff_profile_hook = lambda: holder[0]
        sys.modules["antenv.axon_hooks"] = mod
        antenv.axon_hooks = mod
        mod.set_axon_ntff_profile_hook(
            _ntff_profile_via_ctypes("/opt/axon/libaxon_pjrt.so"))
    except Exception as e:  # profiling is best-effort
        print("ntff shim failed:", e)


def _run(nc, in_maps, trace=False):
    from concourse.bass_utils import run_bass_kernel_spmd
    kw = {}
    if trace:
        _install_ntff_shim()
        kw = dict(trace=True, trace_cores=list(range(NCORES)))
    return run_bass_kernel_spmd(nc, in_maps, list(range(NCORES)), **kw)


def kernel(x, edge_index, W_src, b_src, W_dst, b_dst, _trace=False,
           _return_result=False):
    cfg = _cfg_for(x.shape[0])
    in_maps, plan1, plan2, blocks = _prep_host(
        x, edge_index, W_src, b_src, W_dst, b_dst, cfg)
    nc = _build(cfg, plan1, plan2)
    res = _run(nc, in_maps, trace=_trace)
    out = np.zeros((N, D), np.float32)
    for c in range(NCORES):
        oc = res.results[c]["out"]
        for s, g in enumerate(blocks[c]):
            lo = g * 128
            hi = min(lo + 128, N)
            if lo < N:
                out[lo:hi] = oc[s * 128:s * 128 + (hi - lo)]
    if _return_result:
        return out, res
    return out


# revision 7
# speedup vs baseline: 1.1431x; 1.0087x over previous
"""DirGCNConv on 8 Trainium2 NeuronCores via Bass/Tile (v3: 4-queue gather).

out = (1-a)*(Dout^-1/2 A Din^-1/2 x) @ Wsrc.T + a*(Din^-1/2 A.T Dout^-1/2 x) @ Wdst.T + bias

v3 vs v2:
- dma_gather calls rotate over 4 SWDGE queues; descriptor generation for the
  4 queues runs on 4 distinct Q7 CPU pairs concurrently -> ~4x gather rate
  (2.2ns/idx vs 8.6), removing the GpSimd bottleneck.
- Gathers read raw fp32 x directly (no prescale pass, no bf16 xb/xa DRAM
  round trip): the per-source deg^-1/2 scale is applied to the gathered
  tokens (tokens live on partitions) fused with the fp32->bf16 cast on DVE.
  All degree-derived scales are precomputed host-side (pure edge_index
  metadata, like the rp arrays v2 shipped).
- Dest blocks (128 rows) are assigned to cores by balanced snake dealing and
  sorted within cores, so the SPMD static schedule's max-over-cores padding
  drops from ~12% to ~2%.
"""

import os

import numpy as np
from contextlib import ExitStack

os.environ.setdefault("NEURON_RT_RESET_CORES", "1")

N = 100000
E = 600000
D = 128
NCORES = 8
ALPHA = 0.5

WBOUNDS = [0, 28000, 56000, 84000, 100000]
CALL = 1024          # max tokens per dma_gather call
NQ = 4               # SWDGE queues
NBLK = 98            # dest-block slots per core (8*98=784 >= ceil(N/128)=782)
GBLK = (N + 127) // 128   # 782 real global blocks


def _cfg_for(n_nodes):
    assert n_nodes == WBOUNDS[-1]
    return dict(N=n_nodes, NW=len(WBOUNDS) - 1, NLOC=NBLK * 128, NBLK=NBLK)


def _wrap_idx(arr):
    b = arr.shape[0]
    assert b % 16 == 0
    t = arr.reshape(b // 16, 16).T.copy()
    return np.tile(t, (8, 1)).astype(np.int16)


def _inv_sqrt_np(d):
    return np.where(d > 0, 1.0 / np.sqrt(np.maximum(d, 1.0)), 0.0).astype(
        np.float32)


def _prep_host(x, edge_index, W_src, b_src, W_dst, b_dst, cfg):
    """Index reorganization + degree metadata on host."""
    nw, nblk = cfg["NW"], cfg["NBLK"]
    row = np.asarray(edge_index[0], dtype=np.int64)
    col = np.asarray(edge_index[1], dtype=np.int64)

    deg_out = np.bincount(row, minlength=N).astype(np.float64)
    deg_in = np.bincount(col, minlength=N).astype(np.float64)
    a_full = _inv_sqrt_np(deg_out)      # scale at row endpoint
    b_full = _inv_sqrt_np(deg_in)       # scale at col endpoint

    # --- balanced assignment of global dest blocks to (core, slot) ---
    blk_sz = (np.bincount(row >> 7, minlength=nblk * NCORES)
              + np.bincount(col >> 7, minlength=nblk * NCORES))
    order = np.argsort(-blk_sz, kind="stable")
    blocks = [[] for _ in range(NCORES)]
    for i, g in enumerate(order):
        r = i // NCORES
        c = i % NCORES if r % 2 == 0 else NCORES - 1 - (i % NCORES)
        blocks[c].append(int(g))
    for c in range(NCORES):
        blocks[c].sort(key=lambda g: -blk_sz[g])
        assert len(blocks[c]) == nblk
    # map: global block -> (core, slot)
    blk_core = np.empty(nblk * NCORES, np.int64)
    blk_slot = np.empty(nblk * NCORES, np.int64)
    for c in range(NCORES):
        for s, g in enumerate(blocks[c]):
            blk_core[g] = c
            blk_slot[g] = s

    def bucket(dest, src, sfull):
        """dest-sorted token streams. Returns (plan, per-core arrays)."""
        g_of_d = dest >> 7
        core = blk_core[g_of_d]
        pc = []
        cnt = np.zeros((NCORES, nw, nblk), np.int64)
        for c in range(NCORES):
            m = core == c
            s = src[m].astype(np.int64)
            slot = blk_slot[g_of_d[m]]
            dl = slot * 128 + (dest[m] & 127)      # core-local dest id
            w = np.searchsorted(WBOUNDS, s, side="right") - 1
            o = np.lexsort((dl, slot, w))
            dl, s, w, slot = dl[o], s[o], w[o], slot[o]
            np.add.at(cnt[c], (w, slot), 1)
            pc.append((dl, s, w, slot))
        size_wb = cnt.max(axis=0)                      # [nw, nblk] static
        starts = np.zeros((nw, nblk + 1), np.int64)
        starts[:, 1:] = np.cumsum(size_wb, axis=1)
        wtot = starts[:, -1]
        ntokw = ((wtot + 127) // 128) * 128
        win_tok0 = np.zeros(nw, np.int64)
        win_tok0[1:] = np.cumsum(ntokw)[:-1]
        total = int(ntokw.sum())
        nch_total = total // 128

        windows = []
        for w in range(nw):
            nch = int(ntokw[w]) // 128
            mms = []
            for ci in range(nch):
                lo_t, hi_t = ci * 128, ci * 128 + 128
                for b in range(nblk):
                    if size_wb[w, b] > 0 and starts[w, b] < hi_t \
                            and starts[w, b + 1] > lo_t:
                        mms.append((ci, b))
            calls = [(a, min(CALL, int(ntokw[w]) - a))
                     for a in range(0, int(ntokw[w]), CALL)]
            seg_first, seg_last = {}, {}
            for j, (ci, b) in enumerate(mms):
                seg_first.setdefault(b, j)
                seg_last[b] = j
            windows.append(dict(tok0=int(win_tok0[w]), ntok=int(ntokw[w]),
                                calls=calls, mms=mms,
                                seg_first=seg_first, seg_last=seg_last))

        nmm = sum(len(wd["mms"]) for wd in windows)
        g_list, dl_list, st_list = [], [], []
        for c in range(NCORES):
            dl, s, w, slot = pc[c]
            key = w * nblk + slot
            gs0 = np.r_[0, np.cumsum(np.bincount(key, minlength=nw * nblk))]
            rank = np.arange(len(dl)) - gs0[key]
            pos = win_tok0[w] + starts[w, slot] + rank
            gfull = np.zeros(total, np.int64)          # global src (pad 0)
            dlv = -np.ones(total, np.int64)
            stok = np.zeros(total, np.float32)         # pad tokens scale 0
            gfull[pos] = s
            dlv[pos] = dl
            stok[pos] = sfull[s]
            # window-local int16 gather indices
            gloc = gfull.copy()
            for w2 in range(nw):
                t0, nt = windows[w2]["tok0"], windows[w2]["ntok"]
                gloc[t0:t0 + nt] -= WBOUNDS[w2]
                gloc[t0:t0 + nt] = np.maximum(gloc[t0:t0 + nt], 0)
            import ml_dtypes
            oh = np.zeros((nmm, 128, 128), dtype=np.float32)
            j = 0
            for w2, wd in enumerate(windows):
                dlw = dlv[wd["tok0"]:wd["tok0"] + wd["ntok"]].reshape(-1, 128)
                for (ci, b2) in wd["mms"]:
                    r = dlw[ci] - b2 * 128
                    m = (r >= 0) & (r < 128)
                    oh[j, np.arange(128)[m], r[m]] = 1.0
                    j += 1
            g_list.append(_wrap_idx(gloc.astype(np.int16)))
            dl_list.append(np.ascontiguousarray(
                oh.transpose(1, 0, 2).reshape(128, nmm * 128))
                .astype(ml_dtypes.float8_e4m3))    # [128, nmm*128]
            st_list.append(np.ascontiguousarray(
                stok.reshape(nch_total, 128).T))               # [128, nch]
        return (dict(windows=windows, total=total, nmm=nmm,
                     nch=nch_total), g_list, dl_list, st_list)

    plan1, g1, dl1, st1 = bucket(row, col, b_full)   # agg1[row] += b[col]x[col]
    plan2, g2, dl2, st2 = bucket(col, row, a_full)   # agg2[col] += a[row]x[row]

    # dest-side scales per (core, slot): avec for dir1 (a at dest row),
    # bvec for dir2 (b at dest col); scaled by alpha weights.
    av = np.zeros((NCORES, 128, nblk), np.float32)
    bv = np.zeros((NCORES, 128, nblk), np.float32)
    af_pad = np.r_[a_full, np.zeros(nblk * NCORES * 128 - N, np.float32)]
    bf_pad = np.r_[b_full, np.zeros(nblk * NCORES * 128 - N, np.float32)]
    for c in range(NCORES):
        for s, g in enumerate(blocks[c]):
            av[c, :, s] = (1.0 - ALPHA) * af_pad[g * 128:(g + 1) * 128]
            bv[c, :, s] = ALPHA * bf_pad[g * 128:(g + 1) * 128]

    wsrcT = np.ascontiguousarray(np.asarray(W_src, np.float32).T)
    wdstT = np.ascontiguousarray(np.asarray(W_dst, np.float32).T)
    xf = np.ascontiguousarray(np.asarray(x, np.float32))

    in_maps = []
    for c in range(NCORES):
        in_maps.append({
            "x": xf, "wsrcT": wsrcT, "wdstT": wdstT,
            "bsrc": np.asarray(b_src, np.float32),
            "bdst": np.asarray(b_dst, np.float32),
            "g1": g1[c], "g2": g2[c],
            "dl1": dl1[c], "dl2": dl2[c],
            "st1": st1[c], "st2": st2[c],
            "avec": av[c], "bvec": bv[c],
        })
    return in_maps, plan1, plan2, blocks


def _build(cfg, plan1, plan2):
    import concourse.tile as tile
    from concourse import bacc, mybir

    dt = mybir.dt
    n, nw = cfg["N"], cfg["NW"]
    nloc, nblk = cfg["NLOC"], cfg["NBLK"]

    nc = bacc.Bacc("TRN2", target_bir_lowering=False, debug=False,
                   num_devices=NCORES, num_swdge_queues=NQ)

    x = nc.dram_tensor("x", [n, D], dt.float32, kind="ExternalInput")
    wsrcT = nc.dram_tensor("wsrcT", [D, D], dt.float32, kind="ExternalInput")
    wdstT = nc.dram_tensor("wdstT", [D, D], dt.float32, kind="ExternalInput")
    bsrc = nc.dram_tensor("bsrc", [D], dt.float32, kind="ExternalInput")
    bdst = nc.dram_tensor("bdst", [D], dt.float32, kind="ExternalInput")
    g1 = nc.dram_tensor("g1", [128, plan1["total"] // 16], dt.int16,
                        kind="ExternalInput")
    g2 = nc.dram_tensor("g2", [128, plan2["total"] // 16], dt.int16,
                        kind="ExternalInput")
    dl1 = nc.dram_tensor("dl1", [128, plan1["nmm"] * 128], dt.float8e4,
                         kind="ExternalInput")
    dl2 = nc.dram_tensor("dl2", [128, plan2["nmm"] * 128], dt.float8e4,
                         kind="ExternalInput")
    st1 = nc.dram_tensor("st1", [128, plan1["nch"]], dt.float32,
                         kind="ExternalInput")
    st2 = nc.dram_tensor("st2", [128, plan2["nch"]], dt.float32,
                         kind="ExternalInput")
    avec = nc.dram_tensor("avec", [128, nblk], dt.float32,
                          kind="ExternalInput")
    bvec = nc.dram_tensor("bvec", [128, nblk], dt.float32,
                          kind="ExternalInput")
    out = nc.dram_tensor("out", [nloc, D], dt.float32, kind="ExternalOutput")

    qcnt = [0]

    with tile.TileContext(nc) as tc, ExitStack() as ctx:
        const = ctx.enter_context(tc.tile_pool(name="const", bufs=1))

        wsrcT_f = const.tile([D, D], dt.float32, tag="wsrcf")
        nc.sync.dma_start(wsrcT_f[:], wsrcT.ap())
        wsrcT_sb = const.tile([D, D], dt.bfloat16, tag="wsrc")
        nc.vector.tensor_copy(wsrcT_sb[:], wsrcT_f[:])
        wdstT_f = const.tile([D, D], dt.float32, tag="wdstf")
        nc.sync.dma_start(wdstT_f[:], wdstT.ap())
        wdstT_sb = const.tile([D, D], dt.bfloat16, tag="wdst")
        nc.vector.tensor_copy(wdstT_sb[:], wdstT_f[:])
        brow = const.tile([1, 2 * D], dt.float32, tag="brow")
        nc.sync.dma_start(brow[:, 0:D], bsrc.ap().unsqueeze(0))
        nc.sync.dma_start(brow[:, D:2 * D], bdst.ap().unsqueeze(0))
        bsum = const.tile([1, D], dt.float32, tag="bsum")
        nc.vector.tensor_scalar_mul(bsum[:], brow[:, 0:D], 1.0 - ALPHA)
        bs2 = const.tile([1, D], dt.float32, tag="bs2")
        nc.vector.tensor_scalar_mul(bs2[:], brow[:, D:2 * D], ALPHA)
        nc.vector.tensor_add(bsum[:], bsum[:], bs2[:])
        bias_bc = const.tile([D, D], dt.float32, tag="biasbc")
        nc.gpsimd.partition_broadcast(bias_bc[:], bsum[:])

        g1_sb = const.tile([128, plan1["total"] // 16], dt.int16, tag="g1")
        nc.sync.dma_start(g1_sb[:], g1.ap())
        g2_sb = const.tile([128, plan2["total"] // 16], dt.int16, tag="g2")
        nc.sync.dma_start(g2_sb[:], g2.ap())

        st1_sb = const.tile([128, plan1["nch"]], dt.float32, tag="st1")
        nc.sync.dma_start(st1_sb[:], st1.ap())
        st2_sb = const.tile([128, plan2["nch"]], dt.float32, tag="st2")
        nc.sync.dma_start(st2_sb[:], st2.ap())
        av_sb = const.tile([128, nblk], dt.float32, tag="av")
        nc.sync.dma_start(av_sb[:], avec.ap())
        bv_sb = const.tile([128, nblk], dt.float32, tag="bv")
        nc.sync.dma_start(bv_sb[:], bvec.ap())

        # SBUF accumulators [feat x dest], one per direction
        agg1_sb = const.tile([128, nblk * 128], dt.bfloat16, tag="agg1")
        agg2_sb = const.tile([128, nblk * 128], dt.bfloat16, tag="agg2")
        for agg in (agg1_sb, agg2_sb):
            off = 0
            while off < nblk * 128:
                csz = min(4096, nblk * 128 - off)
                nc.vector.memset(agg[:, off:off + csz], 0.0)
                off += csz

        gpool = ctx.enter_context(tc.tile_pool(name="gat", bufs=8))
        xspool = ctx.enter_context(tc.tile_pool(name="xs", bufs=8))
        spool = ctx.enter_context(tc.tile_pool(name="sb", bufs=8))
        epsum = ctx.enter_context(tc.tile_pool(name="eps", bufs=4,
                                               space="PSUM"))

        def edge_window(plan, w, g_sb, st_sb, dlt, agg_sb, mm0, dtag,
                        final_cb=None):
            wd = plan["windows"][w]
            wbase = WBOUNDS[w]
            xs_src = x.ap()[wbase:WBOUNDS[w + 1], :]
            ch0 = wd["tok0"] // 128
            tiles = {}
            for (a, ln) in wd["calls"]:
                xt = gpool.tile([128, CALL // 128, D], dt.float32,
                                tag="xt" + dtag)
                o = wd["tok0"] + a
                gi = g_sb[:, o // 16:(o + ln) // 16]
                nc.gpsimd.dma_gather(xt[:, 0:ln // 128, :], xs_src, gi,
                                     ln, ln, D, queue_num=qcnt[0] % NQ)
                qcnt[0] += 1
                # fused per-token scale + fp32->bf16 cast
                xs = xspool.tile([128, CALL // 128, D], dt.bfloat16,
                                 tag="xs" + dtag)
                c0 = ch0 + a // 128
                if qcnt[0] % 3 == 0:
                    for g in range(ln // 128):
                        nc.scalar.mul(xs[:, g, :], xt[:, g, :],
                                      st_sb[:, c0 + g:c0 + g + 1])
                else:
                    nc.vector.tensor_tensor(
                        xs[:, 0:ln // 128, :],
                        st_sb[:, c0:c0 + ln // 128].unsqueeze(2).to_broadcast(
                            [128, ln // 128, D]),
                        xt[:, 0:ln // 128, :], mybir.AluOpType.mult)
                for g in range(ln // 128):
                    tiles[a // 128 + g] = (xs, g)
            mms = wd["mms"]
            sb_tiles = []
            for j0 in range(0, len(mms), 8):
                jn = min(8, len(mms) - j0)
                st = spool.tile([128, 8, D], dt.float8e4, tag="st")
                nc.sync.dma_start(
                    st[:, 0:jn, :],
                    dlt.ap()[:, (mm0 + j0) * 128:(mm0 + j0 + jn) * 128]
                    .rearrange("p (j d) -> p j d", d=128))
                sb_tiles.append(st)
            active = {}
            for j, (ci, b) in enumerate(mms):
                xs, g = tiles[ci]
                st = sb_tiles[j // 8]
                if b not in active:
                    active[b] = epsum.tile([128, D], dt.float32, tag="ep",
                                           name="ep")
                ps = active[b]
                nc.tensor.matmul(ps[:], lhsT=xs[:, g, :], rhs=st[:, j % 8, :],
                                 start=(j == wd["seg_first"][b]),
                                 stop=(j == wd["seg_last"][b]))
                if j == wd["seg_last"][b]:
                    with nc.allow_low_precision(reason="bf16 agg staging"):
                        nc.vector.tensor_add(
                            agg_sb[:, b * 128:(b + 1) * 128],
                            agg_sb[:, b * 128:(b + 1) * 128], ps[:])
                    del active[b]
                    if final_cb is not None:
                        final_cb(b)
            return mm0 + len(mms)

        fp = ctx.enter_context(tc.tile_pool(name="fin", bufs=3))
        fps = ctx.enter_context(tc.tile_pool(name="fps", bufs=2,
                                             space="PSUM"))

        def final_block(k):
            ks = slice(k * 128, (k + 1) * 128)
            p1 = fps.tile([128, D], dt.float32, tag="p1", name="p1")
            nc.tensor.matmul(p1[:], lhsT=agg1_sb[:, ks], rhs=wsrcT_sb[:],
                             start=True, stop=True)
            p2 = fps.tile([128, D], dt.float32, tag="p2", name="p2")
            nc.tensor.matmul(p2[:], lhsT=agg2_sb[:, ks], rhs=wdstT_sb[:],
                             start=True, stop=True)
            o1 = fp.tile([128, D], dt.float32, tag="o1", name="o1")
            nc.scalar.mul(o1[:], p1[:], av_sb[:, k:k + 1])
            o2 = fp.tile([128, D], dt.float32, tag="o2", name="o2")
            nc.scalar.mul(o2[:], p2[:], bv_sb[:, k:k + 1])
            fin = fp.tile([128, D], dt.float32, tag="fin", name="fin")
            nc.vector.tensor_add(fin[:], o1[:], o2[:])
            nc.vector.tensor_add(fin[:], fin[:], bias_bc[:])
            nc.sync.dma_start(out.ap()[k * 128:(k + 1) * 128, :], fin[:])

        emitted = set()

        def final_cb(b):
            if b not in emitted:
                emitted.add(b)
                final_block(b)

        mm1, mm2 = 0, 0
        for w in range(nw):
            cb1 = final_cb if w == nw - 1 else None
            mm1 = edge_window(plan1, w, g1_sb, st1_sb, dl1, agg1_sb, mm1,
                              "1", None)
            mm2 = edge_window(plan2, w, g2_sb, st2_sb, dl2, agg2_sb, mm2,
                              "2", cb1)
        for k in range(nblk):
            if k not in emitted:
                emitted.add(k)
                final_block(k)

    nc.compile()
    return nc


def _install_ntff_shim():
    """This image's antenv lacks axon_hooks; inject it so trace=True works."""
    import sys
    import types
    try:
        from antenv import axon_hooks  # noqa: F401
        return
    except ImportError:
        pass
    try:
        import antenv
        from trn_agent_boot.trn_boot import _ntff_profile_via_ctypes
        mod = types.ModuleType("antenv.axon_hooks")
        holder = [None]
        mod.set_axon_ntff_profile_hook = lambda h: holder.__setitem__(0, h)
        mod.get_axon_ntff_profile_hook = lambda: holder[0]
        sys.modules["antenv.axon_hooks"] = mod
        antenv.axon_hooks = mod
        mod.set_axon_ntff_profile_hook(
            _ntff_profile_via_ctypes("/opt/axon/libaxon_pjrt.so"))
    except Exception as e:  # profiling is best-effort
        print("ntff shim failed:", e)


def _run(nc, in_maps, trace=False):
    from concourse.bass_utils import run_bass_kernel_spmd
    kw = {}
    if trace:
        _install_ntff_shim()
        kw = dict(trace=True, trace_cores=list(range(NCORES)))
    return run_bass_kernel_spmd(nc, in_maps, list(range(NCORES)), **kw)


def kernel(x, edge_index, W_src, b_src, W_dst, b_dst, _trace=False,
           _return_result=False):
    cfg = _cfg_for(x.shape[0])
    in_maps, plan1, plan2, blocks = _prep_host(
        x, edge_index, W_src, b_src, W_dst, b_dst, cfg)
    nc = _build(cfg, plan1, plan2)
    res = _run(nc, in_maps, trace=_trace)
    out = np.zeros((N, D), np.float32)
    for c in range(NCORES):
        oc = res.results[c]["out"]
        for s, g in enumerate(blocks[c]):
            lo = g * 128
            hi = min(lo + 128, N)
            if lo < N:
                out[lo:hi] = oc[s * 128:s * 128 + (hi - lo)]
    if _return_result:
        return out, res
    return out


# revision 10
# speedup vs baseline: 1.2477x; 1.0915x over previous
"""DirGCNConv on 8 Trainium2 NeuronCores via Bass/Tile (v3: 4-queue gather).

out = (1-a)*(Dout^-1/2 A Din^-1/2 x) @ Wsrc.T + a*(Din^-1/2 A.T Dout^-1/2 x) @ Wdst.T + bias

v3 vs v2:
- dma_gather calls rotate over 4 SWDGE queues; descriptor generation for the
  4 queues runs on 4 distinct Q7 CPU pairs concurrently -> ~4x gather rate
  (2.2ns/idx vs 8.6), removing the GpSimd bottleneck.
- Gathers read raw fp32 x directly (no prescale pass, no bf16 xb/xa DRAM
  round trip): the per-source deg^-1/2 scale is applied to the gathered
  tokens (tokens live on partitions) fused with the fp32->bf16 cast on DVE.
  All degree-derived scales are precomputed host-side (pure edge_index
  metadata, like the rp arrays v2 shipped).
- Dest blocks (128 rows) are assigned to cores by balanced snake dealing and
  sorted within cores, so the SPMD static schedule's max-over-cores padding
  drops from ~12% to ~2%.
"""

import os

import numpy as np
from contextlib import ExitStack

os.environ.setdefault("NEURON_RT_RESET_CORES", "1")

N = 100000
E = 600000
D = 128
NCORES = 8
ALPHA = 0.5

WBOUNDS = [0, 28000, 56000, 84000, 100000]
CALL = 1024          # max tokens per dma_gather call
NQ = 4               # SWDGE queues
NBLK = 98            # dest-block slots per core (8*98=784 >= ceil(N/128)=782)
GBLK = (N + 127) // 128   # 782 real global blocks


def _cfg_for(n_nodes):
    assert n_nodes == WBOUNDS[-1]
    return dict(N=n_nodes, NW=len(WBOUNDS) - 1, NLOC=NBLK * 128, NBLK=NBLK)


def _wrap_idx(arr):
    b = arr.shape[0]
    assert b % 16 == 0
    t = arr.reshape(b // 16, 16).T.copy()
    return np.tile(t, (8, 1)).astype(np.int16)


def _inv_sqrt_np(d):
    return np.where(d > 0, 1.0 / np.sqrt(np.maximum(d, 1.0)), 0.0).astype(
        np.float32)


def _prep_host(x, edge_index, W_src, b_src, W_dst, b_dst, cfg):
    """Index reorganization + degree metadata on host."""
    nw, nblk = cfg["NW"], cfg["NBLK"]
    row = np.asarray(edge_index[0], dtype=np.int64)
    col = np.asarray(edge_index[1], dtype=np.int64)

    deg_out = np.bincount(row, minlength=N).astype(np.float64)
    deg_in = np.bincount(col, minlength=N).astype(np.float64)
    a_full = _inv_sqrt_np(deg_out)      # scale at row endpoint
    b_full = _inv_sqrt_np(deg_in)       # scale at col endpoint

    # --- balanced assignment of global dest blocks to (core, slot) ---
    blk_sz = (np.bincount(row >> 7, minlength=nblk * NCORES)
              + np.bincount(col >> 7, minlength=nblk * NCORES))
    order = np.argsort(-blk_sz, kind="stable")
    blocks = [[] for _ in range(NCORES)]
    for i, g in enumerate(order):
        r = i // NCORES
        c = i % NCORES if r % 2 == 0 else NCORES - 1 - (i % NCORES)
        blocks[c].append(int(g))
    for c in range(NCORES):
        blocks[c].sort(key=lambda g: -blk_sz[g])
        assert len(blocks[c]) == nblk
    # map: global block -> (core, slot)
    blk_core = np.empty(nblk * NCORES, np.int64)
    blk_slot = np.empty(nblk * NCORES, np.int64)
    for c in range(NCORES):
        for s, g in enumerate(blocks[c]):
            blk_core[g] = c
            blk_slot[g] = s

    def bucket(dest, src, sfull):
        """dest-sorted token streams. Returns (plan, per-core arrays)."""
        g_of_d = dest >> 7
        core = blk_core[g_of_d]
        pc = []
        cnt = np.zeros((NCORES, nw, nblk), np.int64)
        for c in range(NCORES):
            m = core == c
            s = src[m].astype(np.int64)
            slot = blk_slot[g_of_d[m]]
            dl = slot * 128 + (dest[m] & 127)      # core-local dest id
            w = np.searchsorted(WBOUNDS, s, side="right") - 1
            o = np.lexsort((dl, slot, w))
            dl, s, w, slot = dl[o], s[o], w[o], slot[o]
            np.add.at(cnt[c], (w, slot), 1)
            pc.append((dl, s, w, slot))
        size_wb = cnt.max(axis=0)                      # [nw, nblk] static
        starts = np.zeros((nw, nblk + 1), np.int64)
        starts[:, 1:] = np.cumsum(size_wb, axis=1)
        wtot = starts[:, -1]
        ntokw = ((wtot + 127) // 128) * 128
        win_tok0 = np.zeros(nw, np.int64)
        win_tok0[1:] = np.cumsum(ntokw)[:-1]
        total = int(ntokw.sum())
        nch_total = total // 128

        windows = []
        for w in range(nw):
            nch = int(ntokw[w]) // 128
            mms = []
            for ci in range(nch):
                lo_t, hi_t = ci * 128, ci * 128 + 128
                for b in range(nblk):
                    if size_wb[w, b] > 0 and starts[w, b] < hi_t \
                            and starts[w, b + 1] > lo_t:
                        mms.append((ci, b))
            calls = [(a, min(CALL, int(ntokw[w]) - a))
                     for a in range(0, int(ntokw[w]), CALL)]
            seg_first, seg_last = {}, {}
            for j, (ci, b) in enumerate(mms):
                seg_first.setdefault(b, j)
                seg_last[b] = j
            windows.append(dict(tok0=int(win_tok0[w]), ntok=int(ntokw[w]),
                                calls=calls, mms=mms,
                                seg_first=seg_first, seg_last=seg_last))

        nmm = sum(len(wd["mms"]) for wd in windows)
        g_list, dl_list, st_list = [], [], []
        for c in range(NCORES):
            dl, s, w, slot = pc[c]
            key = w * nblk + slot
            gs0 = np.r_[0, np.cumsum(np.bincount(key, minlength=nw * nblk))]
            rank = np.arange(len(dl)) - gs0[key]
            pos = win_tok0[w] + starts[w, slot] + rank
            gfull = np.zeros(total, np.int64)          # global src (pad 0)
            dlv = -np.ones(total, np.int64)
            stok = np.zeros(total, np.float32)         # pad tokens scale 0
            gfull[pos] = s
            dlv[pos] = dl
            stok[pos] = sfull[s]
            # window-local int16 gather indices
            gloc = gfull.copy()
            for w2 in range(nw):
                t0, nt = windows[w2]["tok0"], windows[w2]["ntok"]
                gloc[t0:t0 + nt] -= WBOUNDS[w2]
                gloc[t0:t0 + nt] = np.maximum(gloc[t0:t0 + nt], 0)
            import ml_dtypes
            oh = np.zeros((nmm, 128, 128), dtype=np.float32)
            j = 0
            for w2, wd in enumerate(windows):
                dlw = dlv[wd["tok0"]:wd["tok0"] + wd["ntok"]].reshape(-1, 128)
                for (ci, b2) in wd["mms"]:
                    r = dlw[ci] - b2 * 128
                    m = (r >= 0) & (r < 128)
                    oh[j, np.arange(128)[m], r[m]] = 1.0
                    j += 1
            g_list.append(_wrap_idx(gloc.astype(np.int16)))
            dl_list.append(np.ascontiguousarray(
                oh.transpose(1, 0, 2).reshape(128, nmm * 128))
                .astype(ml_dtypes.float8_e4m3))    # [128, nmm*128]
            st_list.append(np.ascontiguousarray(
                stok.reshape(nch_total, 128).T))               # [128, nch]
        return (dict(windows=windows, total=total, nmm=nmm,
                     nch=nch_total), g_list, dl_list, st_list)

    plan1, g1, dl1, st1 = bucket(row, col, b_full)   # agg1[row] += b[col]x[col]
    plan2, g2, dl2, st2 = bucket(col, row, a_full)   # agg2[col] += a[row]x[row]

    # dest-side scales per (core, slot): avec for dir1 (a at dest row),
    # bvec for dir2 (b at dest col); scaled by alpha weights.
    av = np.zeros((NCORES, 128, nblk), np.float32)
    bv = np.zeros((NCORES, 128, nblk), np.float32)
    af_pad = np.r_[a_full, np.zeros(nblk * NCORES * 128 - N, np.float32)]
    bf_pad = np.r_[b_full, np.zeros(nblk * NCORES * 128 - N, np.float32)]
    for c in range(NCORES):
        for s, g in enumerate(blocks[c]):
            av[c, :, s] = (1.0 - ALPHA) * af_pad[g * 128:(g + 1) * 128]
            bv[c, :, s] = ALPHA * bf_pad[g * 128:(g + 1) * 128]

    wsrcT = np.ascontiguousarray(np.asarray(W_src, np.float32).T)
    wdstT = np.ascontiguousarray(np.asarray(W_dst, np.float32).T)
    xf = np.ascontiguousarray(np.asarray(x, np.float32))

    in_maps = []
    for c in range(NCORES):
        in_maps.append({
            "x": xf, "wsrcT": wsrcT, "wdstT": wdstT,
            "bsrc": np.asarray(b_src, np.float32),
            "bdst": np.asarray(b_dst, np.float32),
            "g1": g1[c], "g2": g2[c],
            "dl1": dl1[c], "dl2": dl2[c],
            "st1": st1[c], "st2": st2[c],
            "avec": av[c], "bvec": bv[c],
        })
    return in_maps, plan1, plan2, blocks


def _build(cfg, plan1, plan2):
    import concourse.tile as tile
    from concourse import bacc, mybir

    dt = mybir.dt
    n, nw = cfg["N"], cfg["NW"]
    nloc, nblk = cfg["NLOC"], cfg["NBLK"]

    nc = bacc.Bacc("TRN2", target_bir_lowering=False, debug=False,
                   num_devices=NCORES, num_swdge_queues=NQ)

    x = nc.dram_tensor("x", [n, D], dt.float32, kind="ExternalInput")
    wsrcT = nc.dram_tensor("wsrcT", [D, D], dt.float32, kind="ExternalInput")
    wdstT = nc.dram_tensor("wdstT", [D, D], dt.float32, kind="ExternalInput")
    bsrc = nc.dram_tensor("bsrc", [D], dt.float32, kind="ExternalInput")
    bdst = nc.dram_tensor("bdst", [D], dt.float32, kind="ExternalInput")
    g1 = nc.dram_tensor("g1", [128, plan1["total"] // 16], dt.int16,
                        kind="ExternalInput")
    g2 = nc.dram_tensor("g2", [128, plan2["total"] // 16], dt.int16,
                        kind="ExternalInput")
    dl1 = nc.dram_tensor("dl1", [128, plan1["nmm"] * 128], dt.float8e4,
                         kind="ExternalInput")
    dl2 = nc.dram_tensor("dl2", [128, plan2["nmm"] * 128], dt.float8e4,
                         kind="ExternalInput")
    st1 = nc.dram_tensor("st1", [128, plan1["nch"]], dt.float32,
                         kind="ExternalInput")
    st2 = nc.dram_tensor("st2", [128, plan2["nch"]], dt.float32,
                         kind="ExternalInput")
    avec = nc.dram_tensor("avec", [128, nblk], dt.float32,
                          kind="ExternalInput")
    bvec = nc.dram_tensor("bvec", [128, nblk], dt.float32,
                          kind="ExternalInput")
    out = nc.dram_tensor("out", [nloc, D], dt.float32, kind="ExternalOutput")

    qcnt = [0]

    with tile.TileContext(nc) as tc, ExitStack() as ctx:
        const = ctx.enter_context(tc.tile_pool(name="const", bufs=1))

        wsrcT_f = const.tile([D, D], dt.float32, tag="wsrcf")
        nc.sync.dma_start(wsrcT_f[:], wsrcT.ap())
        wsrcT_sb = const.tile([D, D], dt.bfloat16, tag="wsrc")
        nc.vector.tensor_copy(wsrcT_sb[:], wsrcT_f[:])
        wdstT_f = const.tile([D, D], dt.float32, tag="wdstf")
        nc.sync.dma_start(wdstT_f[:], wdstT.ap())
        wdstT_sb = const.tile([D, D], dt.bfloat16, tag="wdst")
        nc.vector.tensor_copy(wdstT_sb[:], wdstT_f[:])
        brow = const.tile([1, 2 * D], dt.float32, tag="brow")
        nc.sync.dma_start(brow[:, 0:D], bsrc.ap().unsqueeze(0))
        nc.sync.dma_start(brow[:, D:2 * D], bdst.ap().unsqueeze(0))
        bsum = const.tile([1, D], dt.float32, tag="bsum")
        nc.vector.tensor_scalar_mul(bsum[:], brow[:, 0:D], 1.0 - ALPHA)
        bs2 = const.tile([1, D], dt.float32, tag="bs2")
        nc.vector.tensor_scalar_mul(bs2[:], brow[:, D:2 * D], ALPHA)
        nc.vector.tensor_add(bsum[:], bsum[:], bs2[:])
        bias_bc = const.tile([D, D], dt.float32, tag="biasbc")
        nc.gpsimd.partition_broadcast(bias_bc[:], bsum[:])

        g1_sb = const.tile([128, plan1["total"] // 16], dt.int16, tag="g1")
        nc.sync.dma_start(g1_sb[:], g1.ap())
        g2_sb = const.tile([128, plan2["total"] // 16], dt.int16, tag="g2")
        nc.sync.dma_start(g2_sb[:], g2.ap())

        st1_sb = const.tile([128, plan1["nch"]], dt.float32, tag="st1")
        nc.sync.dma_start(st1_sb[:], st1.ap())
        st2_sb = const.tile([128, plan2["nch"]], dt.float32, tag="st2")
        nc.sync.dma_start(st2_sb[:], st2.ap())
        av_sb = const.tile([128, nblk], dt.float32, tag="av")
        nc.sync.dma_start(av_sb[:], avec.ap())
        bv_sb = const.tile([128, nblk], dt.float32, tag="bv")
        nc.sync.dma_start(bv_sb[:], bvec.ap())

        # SBUF accumulators [feat x dest], one per direction
        agg1_sb = const.tile([128, nblk * 128], dt.bfloat16, tag="agg1")
        agg2_sb = const.tile([128, nblk * 128], dt.bfloat16, tag="agg2")
        for agg in (agg1_sb, agg2_sb):
            off = 0
            while off < nblk * 128:
                csz = min(4096, nblk * 128 - off)
                nc.vector.memset(agg[:, off:off + csz], 0.0)
                off += csz

        gpool = ctx.enter_context(tc.tile_pool(name="gat", bufs=8))
        xspool = ctx.enter_context(tc.tile_pool(name="xs", bufs=8))
        spool = ctx.enter_context(tc.tile_pool(name="sb", bufs=8))
        epsum = ctx.enter_context(tc.tile_pool(name="eps", bufs=6,
                                               space="PSUM"))

        def emit_calls(plan, w, g_sb, st_sb, dtag, tiles, calls_slice):
            wd = plan["windows"][w]
            wbase = WBOUNDS[w]
            xs_src = x.ap()[wbase:WBOUNDS[w + 1], :]
            ch0 = wd["tok0"] // 128
            for (a, ln) in calls_slice:
                xt = gpool.tile([128, CALL // 128, D], dt.float32,
                                tag="xt" + dtag)
                o = wd["tok0"] + a
                gi = g_sb[:, o // 16:(o + ln) // 16]
                nc.gpsimd.dma_gather(xt[:, 0:ln // 128, :], xs_src, gi,
                                     ln, ln, D, queue_num=qcnt[0] % NQ)
                qcnt[0] += 1
                # fused per-token scale + fp32->bf16 cast
                xs = xspool.tile([128, CALL // 128, D], dt.bfloat16,
                                 tag="xs" + dtag)
                c0 = ch0 + a // 128
                if qcnt[0] % 3 == 0:
                    for g in range(ln // 128):
                        nc.scalar.mul(xs[:, g, :], xt[:, g, :],
                                      st_sb[:, c0 + g:c0 + g + 1])
                else:
                    nc.vector.tensor_tensor(
                        xs[:, 0:ln // 128, :],
                        st_sb[:, c0:c0 + ln // 128].unsqueeze(2).to_broadcast(
                            [128, ln // 128, D]),
                        xt[:, 0:ln // 128, :], mybir.AluOpType.mult)
                for g in range(ln // 128):
                    tiles[a // 128 + g] = (xs, g)

        def edge_mms(plan, w, dlt, agg_sb, mm0, tiles, final_cb=None):
            wd = plan["windows"][w]
            mms = wd["mms"]
            sb_tiles = []
            for j0 in range(0, len(mms), 8):
                jn = min(8, len(mms) - j0)
                st = spool.tile([128, 8, D], dt.float8e4, tag="st")
                nc.sync.dma_start(
                    st[:, 0:jn, :],
                    dlt.ap()[:, (mm0 + j0) * 128:(mm0 + j0 + jn) * 128]
                    .rearrange("p (j d) -> p j d", d=128))
                sb_tiles.append(st)
            active = {}
            for j, (ci, b) in enumerate(mms):
                xs, g = tiles[ci]
                st = sb_tiles[j // 8]
                if b not in active:
                    active[b] = epsum.tile([128, D], dt.float32, tag="ep",
                                           name="ep")
                ps = active[b]
                nc.tensor.matmul(ps[:], lhsT=xs[:, g, :], rhs=st[:, j % 8, :],
                                 start=(j == wd["seg_first"][b]),
                                 stop=(j == wd["seg_last"][b]))
                if j == wd["seg_last"][b]:
                    with nc.allow_low_precision(reason="bf16 agg staging"):
                        nc.vector.tensor_add(
                            agg_sb[:, b * 128:(b + 1) * 128],
                            agg_sb[:, b * 128:(b + 1) * 128], ps[:])
                    del active[b]
                    if final_cb is not None:
                        final_cb(b)
            return mm0 + len(mms)

        fp = ctx.enter_context(tc.tile_pool(name="fin", bufs=3))
        fps = ctx.enter_context(tc.tile_pool(name="fps", bufs=2,
                                             space="PSUM"))

        def final_block(k):
            ks = slice(k * 128, (k + 1) * 128)
            pf = fps.tile([128, 2, D], dt.float32, tag="pf", name="pf")
            nc.tensor.matmul(pf[:, 0, :], lhsT=agg1_sb[:, ks],
                             rhs=wsrcT_sb[:], start=True, stop=True)
            nc.tensor.matmul(pf[:, 1, :], lhsT=agg2_sb[:, ks],
                             rhs=wdstT_sb[:], start=True, stop=True)
            o1 = fp.tile([128, D], dt.float32, tag="o1", name="o1")
            nc.scalar.mul(o1[:], pf[:, 0, :], av_sb[:, k:k + 1])
            o2 = fp.tile([128, D], dt.float32, tag="o2", name="o2")
            nc.scalar.mul(o2[:], pf[:, 1, :], bv_sb[:, k:k + 1])
            fin = fp.tile([128, D], dt.float32, tag="fin", name="fin")
            nc.vector.tensor_add(fin[:], o1[:], o2[:])
            nc.vector.tensor_add(fin[:], fin[:], bias_bc[:])
            nc.sync.dma_start(out.ap()[k * 128:(k + 1) * 128, :], fin[:])

        emitted = set()

        def final_cb(b):
            if b not in emitted:
                emitted.add(b)
                final_block(b)

        # segment list: (plan, g, st, dl, agg, dtag, w, cb)
        segs = []
        for w in range(nw):
            segs.append((plan1, g1_sb, st1_sb, dl1, agg1_sb, "1", w, None))
            segs.append((plan2, g2_sb, st2_sb, dl2, agg2_sb, "2", w,
                         final_cb if w == nw - 1 else None))
        PRE = 6
        tiles_of = [dict() for _ in segs]
        mmof = {"1": 0, "2": 0}
        for i, (pl, g_sb, st_sb, dlt, agg, dtag, w, cb) in enumerate(segs):
            calls = pl["windows"][w]["calls"]
            if i == 0:
                emit_calls(pl, w, g_sb, st_sb, dtag, tiles_of[i], calls[:PRE])
            # pre-issue next segment's head before this segment's mms
            emit_calls(pl, w, g_sb, st_sb, dtag, tiles_of[i], calls[PRE:])
            if i + 1 < len(segs):
                pl2, g2s, st2s, dlt2, agg2t, dt2, w2, _ = segs[i + 1]
                emit_calls(pl2, w2, g2s, st2s, dt2, tiles_of[i + 1],
                           pl2["windows"][w2]["calls"][:PRE])
            mmof[dtag] = edge_mms(pl, w, dlt, agg, mmof[dtag],
                                  tiles_of[i], cb)
        for k in range(nblk):
            if k not in emitted:
                emitted.add(k)
                final_block(k)

    nc.compile()
    return nc


def _install_ntff_shim():
    """This image's antenv lacks axon_hooks; inject it so trace=True works."""
    import sys
    import types
    try:
        from antenv import axon_hooks  # noqa: F401
        return
    except ImportError:
        pass
    try:
        import antenv
        from trn_agent_boot.trn_boot import _ntff_profile_via_ctypes
        mod = types.ModuleType("antenv.axon_hooks")
        holder = [None]
        mod.set_axon_ntff_profile_hook = lambda h: holder.__setitem__(0, h)
        mod.get_axon_ntff_profile_hook = lambda: holder[0]
        sys.modules["antenv.axon_hooks"] = mod
        antenv.axon_hooks = mod
        mod.set_axon_ntff_profile_hook(
            _ntff_profile_via_ctypes("/opt/axon/libaxon_pjrt.so"))
    except Exception as e:  # profiling is best-effort
        print("ntff shim failed:", e)


def _run(nc, in_maps, trace=False):
    from concourse.bass_utils import run_bass_kernel_spmd
    kw = {}
    if trace:
        _install_ntff_shim()
        kw = dict(trace=True, trace_cores=list(range(NCORES)))
    return run_bass_kernel_spmd(nc, in_maps, list(range(NCORES)), **kw)


def kernel(x, edge_index, W_src, b_src, W_dst, b_dst, _trace=False,
           _return_result=False):
    cfg = _cfg_for(x.shape[0])
    in_maps, plan1, plan2, blocks = _prep_host(
        x, edge_index, W_src, b_src, W_dst, b_dst, cfg)
    nc = _build(cfg, plan1, plan2)
    res = _run(nc, in_maps, trace=_trace)
    out = np.zeros((N, D), np.float32)
    for c in range(NCORES):
        oc = res.results[c]["out"]
        for s, g in enumerate(blocks[c]):
            lo = g * 128
            hi = min(lo + 128, N)
            if lo < N:
                out[lo:hi] = oc[s * 128:s * 128 + (hi - lo)]
    if _return_result:
        return out, res
    return out
